# revision 32
# baseline (speedup 1.0000x reference)
"""Trainium2 Bass kernel for nn_LonelyDecoder (dense transformer, 8-core TP).

v6 highlights:
 - fp16 everywhere (same PE rate as bf16, ~8x less rounding noise).
 - embW resident in SBUF; per-s-chunk activation tiles (no whole-tile
   false deps across pipeline stages).
 - One software-pipelined schedule with lag-2 stages: MHA2 chunk sc |
   ln2 chunk sc-1 | FFN chunk sc-2, and ln3(sc+1) injected mid
   output-GEMM(sc), so the LN scalar chain (DVE smalls + gpsimd
   broadcast) never sits on the PE critical path.
 - LN rsqrt computed on DVE via bitcast+Newton (no ACT Sqrt table-set
   switches); all broadcasts via gpsimd PartitionBroadcast; gpsimd kept
   extended-lib-only (library reloads cost ~8us).
 - Output GEMM per s-chunk with pipelined softmax-sum AllReduce; only
   the last chunk's AR + normalize + writeout is exposed.
"""

import numpy as np
import ml_dtypes

import concourse.bacc as bacc
import concourse.bass as bass
import concourse.mybir as mybir
import concourse.tile as tile
from concourse.bass_utils import run_bass_kernel_spmd

F32 = mybir.dt.float32
F16 = mybir.dt.float16
I32 = mybir.dt.int32
AF = mybir.ActivationFunctionType
ALU = mybir.AluOpType

S, V, D, H, DK, DFF, L = 2048, 32000, 1024, 16, 64, 4096, 4
NCORES = 8
VSR = V // NCORES          # 4000 real vocab shard
VSP = 4096                 # padded vocab shard (32 x 128)
NVC = VSP // 128           # 32 v-chunks
NDC = D // 128             # 8 d-chunks
NSC = 4                    # s-chunks of 512
SC = 512
NTT = S // 128             # 16 t-tiles
FS = DFF // NCORES         # 512 ff shard
NFC = FS // 128            # 4 ff chunks
RG = [list(range(NCORES))]

LAST_RESULTS = {}


def ts(i, n):
    return slice(i * n, (i + 1) * n)


def build_bass():
    nc = bacc.Bacc(None, target_bir_lowering=False)

    # ---- I/O ----
    xT = nc.dram_tensor("xT", [VSP, S], F16, kind="ExternalInput")
    embW = nc.dram_tensor("embW", [VSP, D], F16, kind="ExternalInput")
    pebT = nc.dram_tensor("pebT", [D, S], F16, kind="ExternalInput")
    qkw = [nc.dram_tensor(f"qkw{m}", [128, NDC, 256], F16, kind="ExternalInput") for m in (1, 2)]
    bqk = [nc.dram_tensor(f"bqk{m}", [128, 2], F32, kind="ExternalInput") for m in (1, 2)]
    vw = [nc.dram_tensor(f"vw{m}", [128, NDC, 130], F16, kind="ExternalInput") for m in (1, 2)]
    bv = [nc.dram_tensor(f"bv{m}", [128, 1], F32, kind="ExternalInput") for m in (1, 2)]
    maskT = nc.dram_tensor("maskT", [128, 4 * SC], F16, kind="ExternalInput")
    f1w = nc.dram_tensor("f1w", [128, NDC, FS], F16, kind="ExternalInput")
    f1b = nc.dram_tensor("f1b", [128, NFC], F32, kind="ExternalInput")
    f2w = nc.dram_tensor("f2w", [128, NFC, D], F16, kind="ExternalInput")
    f2bT = nc.dram_tensor("f2bT", [128, NDC], F32, kind="ExternalInput")
    lngT = nc.dram_tensor("lngT", [128, NDC], F32, kind="ExternalInput")
    lnbT = nc.dram_tensor("lnbT", [128, NDC], F32, kind="ExternalInput")
    outw = nc.dram_tensor("outw", [NVC, 128, NDC, 128], F16, kind="ExternalInput")
    outb = nc.dram_tensor("outb", [128, NVC], F32, kind="ExternalInput")
    probsT = nc.dram_tensor("probsT", [VSP, S], F16, kind="ExternalOutput")

    with tile.TileContext(nc) as tc:
        with tc.tile_pool(name="dram", bufs=1, space="DRAM") as dram, \
             tc.tile_pool(name="const", bufs=1) as const:

            # internal DRAM (collective bounce buffers)
            h_par = [dram.tile([D, SC], F16, tag=f"hp{sc}", name=f"h_par{sc}")
                     for sc in range(NSC)]
            h_red = [dram.tile([D, SC], F16, tag=f"hr{sc}", addr_space="Shared",
                               name=f"h_red{sc}") for sc in range(NSC)]
            a_in = [[dram.tile([128, SC], F16, tag=f"a{m}i{sc}", name=f"a{m}_in{sc}")
                     for sc in range(NSC)] for m in (0, 1)]
            a_out = [[dram.tile([D, SC], F16, tag=f"a{m}o{sc}", addr_space="Shared",
                                name=f"a{m}_out{sc}") for sc in range(NSC)]
                     for m in (0, 1)]
            y_par = [dram.tile([D, SC], F16, tag=f"yp{sc}", name=f"y_par{sc}")
                     for sc in range(NSC)]
            y_red = [dram.tile([D, SC], F16, tag=f"yr{sc}", addr_space="Shared",
                               name=f"y_red{sc}") for sc in range(NSC)]
            ss_in = [dram.tile([1, SC], F32, tag=f"ssi{sc}", name=f"ss_in{sc}")
                     for sc in range(NSC)]
            ss_out = [dram.tile([1, SC], F32, tag=f"sso{sc}", addr_space="Shared",
                                name=f"ss_out{sc}") for sc in range(NSC)]

            # constants (scalar DMA queue; sync stays free for GEMM tiles)
            ones_col = const.tile([128, 1], F16, tag="c1")
            nc.vector.memset(ones_col[:, :], 1.0)
            ones_row = const.tile([1, 128], F16, tag="c3")
            nc.vector.memset(ones_row[:, :], 1.0)
            bqk_sb = [const.tile([128, 2], F32, tag=f"bqk{m}", name=f"bqk_sb{m}") for m in range(2)]
            bv_sb = [const.tile([128, 1], F32, tag=f"bv{m}", name=f"bv_sb{m}") for m in range(2)]
            for m in range(2):
                nc.scalar.dma_start(bqk_sb[m][:, :], bqk[m][:, :])
                nc.scalar.dma_start(bv_sb[m][:, :], bv[m][:, :])
            f1b_sb = const.tile([128, NFC], F32, tag="f1b")
            nc.scalar.dma_start(f1b_sb[:, :], f1b[:, :])
            f2bT_sb = const.tile([128, NDC], F32, tag="f2bT")
            nc.scalar.dma_start(f2bT_sb[:, :], f2bT[:, :])
            lng_sb = const.tile([128, NDC], F32, tag="lng")
            nc.scalar.dma_start(lng_sb[:, :], lngT[:, :])
            lnb_sb = const.tile([128, NDC], F32, tag="lnb")
            nc.scalar.dma_start(lnb_sb[:, :], lnbT[:, :])
            outb_sb = const.tile([128, NVC], F32, tag="outb")
            nc.scalar.dma_start(outb_sb[:, :], outb[:, :])

            # per-s-chunk activation tiles: tag per sc so cross-chunk
            # consumers never pick up whole-tile false dependencies.
            acts_ctx = tc.tile_pool(name="acts", bufs=2)
            acts = acts_ctx.__enter__()

            def act_tiles(name):
                return [acts.tile([128, NDC, SC], F16, tag=f"act{sc}",
                                  name=f"{name}{sc}") for sc in range(NSC)]

            hT = act_tiles("hT")

            # ---------- phase E: embedding GEMM (chunked AllReduce) ----------
            with tc.tile_pool(name="embw", bufs=1) as embp, \
                 tc.tile_pool(name="xt", bufs=6) as xtp, \
                 tc.tile_pool(name="peb", bufs=1) as pebp, \
                 tc.tile_pool(name="ps_e", bufs=1, space="PSUM") as pse, \
                 tc.tile_pool(name="ev_e", bufs=3) as evp, \
                 tc.tile_pool(name="addin_e", bufs=3) as adpe:
                # embW resident (64KB/partition), loaded once on scalar queue
                ew_sb = embp.tile([128, NVC, D], F16, tag="ew")
                peb_sb = pebp.tile([128, NDC, S], F16, tag="peb")
                for dc in range(NDC):
                    nc.gpsimd.dma_start(peb_sb[:, dc, :], pebT[ts(dc, 128), :])

                def ht_prep(psc):
                    for dc in range(NDC):
                        hr = adpe.tile([128, SC], F16, tag="addin",
                                       name=f"hr_{psc}_{dc}")
                        nc.scalar.dma_start(hr[:, :], h_red[psc][ts(dc, 128), :])
                        nc.vector.tensor_add(hT[psc][:, dc, :], hr[:, :],
                                             peb_sb[:, dc, ts(psc, SC)])

                for sc in range(NSC):
                    with nc.named_scope(f"E{sc}"):
                        pes = [pse.tile([128, SC], F32, tag=f"pe{dc}",
                                        name=f"pe_{sc}_{dc}")
                               for dc in range(NDC)]
                        for kc in range(NVC):
                            xt = xtp.tile([128, SC], F16, tag="xt")
                            nc.sync.dma_start(xt[:, :], xT[ts(kc, 128), ts(sc, SC)])
                            if sc == 0:
                                nc.scalar.dma_start(ew_sb[:, kc, :],
                                                    embW[ts(kc, 128), :])
                            for dc in range(NDC):
                                nc.tensor.matmul(
                                    pes[dc][:, :],
                                    ew_sb[:, kc, ts(dc, 128)],
                                    xt[:, :],
                                    start=(kc == 0),
                                    stop=(kc == NVC - 1),
                                )
                        for dc in range(NDC):
                            hv = evp.tile([128, SC], F16, tag="ev")
                            nc.scalar.activation(hv[:, :], pes[dc][:, :], AF.Copy)
                            nc.sync.dma_start(h_par[sc][ts(dc, 128), :], hv[:, :])
                        nc.gpsimd.collective_compute(
                            "AllReduce", ALU.add, replica_groups=RG,
                            ins=[h_par[sc][:, :].opt()],
                            outs=[h_red[sc][:, :].opt()],
                        )
                        if sc > 0:
                            ht_prep(sc - 1)
                ht_prep(NSC - 1)

            # ======== pipelined layer ========
            # unified PSUM pool (8 banks): ps1(2) + po(2) + pg(2x2=4)
            with tc.tile_pool(name="addin", bufs=3) as adp, \
                 tc.tile_pool(name="x2p", bufs=2) as x2p, \
                 tc.tile_pool(name="ev_a", bufs=2) as evp, \
                 tc.tile_pool(name="small_a", bufs=2) as smp, \
                 tc.tile_pool(name="osc", bufs=2) as osc, \
                 tc.tile_pool(name="pp", bufs=3) as ppp, \
                 tc.tile_pool(name="outwp", bufs=2) as owp, \
                 tc.tile_pool(name="ffw", bufs=1) as ffp, \
                 tc.tile_pool(name="exp", bufs=2) as expp, \
                 tc.tile_pool(name="ps_a", bufs=2, space="PSUM") as psa:

                def mha_proj_chunk(mi, sc, actT, qkw_sb, vw_sb, V_sb, qT2, kT2):
                    """QKV projections for weight-set mi, one s-chunk."""
                    for tt in range(4 * sc, 4 * sc + 4):
                        pv = psa.tile([128, SC], F32, tag="ps1", name=f"pv{mi}_{tt}")
                        for dc in range(NDC):
                            nc.tensor.matmul(
                                pv[:, 0:130], actT[sc][:, dc, ts(tt % 4, 128)],
                                vw_sb[:, dc, :],
                                start=(dc == 0), stop=(dc == NDC - 1),
                            )
                        nc.scalar.activation(V_sb[:, tt, :], pv[:, 0:130], AF.Copy)
                    for wi, dst in ((0, qT2), (1, kT2)):
                        pq = psa.tile([128, SC], F32, tag="ps1", name=f"pq{mi}_{wi}_{sc}")
                        for dc in range(NDC):
                            nc.tensor.matmul(
                                pq[:, :],
                                qkw_sb[:, dc, ts(wi, 128)],
                                actT[sc][:, dc, :],
                                start=(dc == 0), stop=(dc == NDC - 1),
                            )
                        nc.scalar.activation(
                            dst[:, sc, :], pq[:, :], AF.Identity,
                            bias=bqk_sb[mi][:, wi:wi + 1],
                        )
                    nc.vector.memset(V_sb[:, ts(sc, 4), 64:65], 1.0)
                    nc.vector.memset(V_sb[:, ts(sc, 4), 129:130], 1.0)

                def mha_chunk(mi, sc, masked, attnT, V_sb, qT2, kT2, mask_sb):
                    """scores+AV+normalize for one s-chunk, both heads, then
                    a_in DMA + AllGather."""
                    for h in range(2):
                        po = psa.tile([128, SC], F32, tag="po", name=f"po{mi}_{h}_{sc}")
                        tts = list(range(4 * (sc + 1))) if masked else list(range(NTT))
                        pairs = [tts[i:i + 2] for i in range(0, len(tts), 2)]
                        for pi, pr in enumerate(pairs):
                            pg = psa.tile([128, 2 * SC], F32, tag="pg",
                                          name=f"pg{mi}_{h}_{sc}_{pi}")
                            for j, tt in enumerate(pr):
                                nc.tensor.matmul(
                                    pg[:, ts(j, SC)],
                                    kT2[ts(h, 64), tt // 4, ts(tt % 4, 128)],
                                    qT2[ts(h, 64), sc, :],
                                    start=True, stop=True,
                                )
                            et = evp.tile([128, 2 * SC], F16, tag="exp")
                            nc.scalar.activation(et[:, :], pg[:, :], AF.Exp,
                                                 scale=1.0 / D)
                            if masked and pr[0] >= 4 * sc:
                                mo = (pr[0] - 4 * sc) * SC
                                nc.vector.tensor_mul(
                                    et[:, :], et[:, :],
                                    mask_sb[:, mo:mo + 2 * SC],
                                )
                            for j, tt in enumerate(pr):
                                nc.tensor.matmul(
                                    po[0:65, :],
                                    V_sb[:, tt, ts(h, 65)],
                                    et[:, ts(j, SC)],
                                    start=(pi == 0 and j == 0),
                                    stop=(pi == len(pairs) - 1 and j == len(pr) - 1),
                                )
                        oo = smp.tile([64, SC], F16, tag="oo", bufs=1, name=f"oo{mi}_{h}_{sc}")
                        nc.scalar.activation(oo[:, :], po[0:64, :], AF.Copy)
                        s0 = smp.tile([1, SC], F32, tag="s0", bufs=1, name=f"s0{mi}_{h}_{sc}")
                        nc.scalar.activation(s0[:, :], po[64:65, :], AF.Copy)
                        rec = smp.tile([1, SC], F32, tag="rec", bufs=1, name=f"rec{mi}_{h}_{sc}")
                        nc.vector.reciprocal_approx_fast(rec[:, :], s0[:, :])
                        rec16 = smp.tile([1, SC], F16, tag="rec16", bufs=1, name=f"rec16_{mi}_{h}_{sc}")
                        nc.vector.tensor_copy(rec16[:, :], rec[:, :])
                        rb = smp.tile([64, SC], F16, tag="rb", bufs=1, name=f"rb{mi}_{h}_{sc}")
                        nc.gpsimd.partition_broadcast(rb[:, :], rec16[0:1, :])
                        nc.vector.tensor_mul(oo[:, :], oo[:, :], rb[:, :])
                        nc.scalar.activation(
                            attnT[ts(h, 64), sc, :], oo[:, :], AF.Identity,
                            bias=bv_sb[mi][ts(h, 64), :],
                        )
                    nc.sync.dma_start(a_in[mi][sc][:, :], attnT[:, sc, :])
                    nc.gpsimd.collective_compute(
                        "AllGather", ALU.bypass, replica_groups=RG,
                        ins=[a_in[mi][sc][:, :].opt()],
                        outs=[a_out[mi][sc][:, :].opt()],
                    )

                # residual + layernorm over the feature dim for ONE s-chunk.
                # prevT/newT are lists of per-sc tiles [128, NDC, SC].
                def ln_chunk(prevT, newT, sc, addin_fn, name):
                    stats = psa.tile([65, SC], F32, tag="ps1",
                                     name=f"st_{name}_{sc}")
                    for dc in range(NDC):
                        src_ap, xbias = addin_fn(sc, dc)
                        ad = adp.tile([128, SC], F16, tag="addin",
                                      name=f"ad_{name}_{sc}_{dc}")
                        nc.scalar.dma_start(ad[:, :], src_ap)
                        if xbias is not None:
                            nc.vector.scalar_tensor_tensor(
                                prevT[sc][:, dc, :], ad[:, :], xbias,
                                prevT[sc][:, dc, :], op0=ALU.add, op1=ALU.add)
                        else:
                            nc.vector.tensor_add(prevT[sc][:, dc, :],
                                                 prevT[sc][:, dc, :], ad[:, :])
                        x2 = x2p.tile([128, SC], F16, tag="x2",
                                      name=f"x2_{name}_{sc}_{dc}")
                        nc.vector.tensor_mul(x2[:, :], prevT[sc][:, dc, :],
                                             prevT[sc][:, dc, :])
                        nc.tensor.matmul(stats[0:1, :], ones_col[:, :],
                                         prevT[sc][:, dc, :],
                                         start=(dc == 0), stop=(dc == NDC - 1))
                        nc.tensor.matmul(stats[64:65, :], ones_col[:, :],
                                         x2[:, :],
                                         start=(dc == 0), stop=(dc == NDC - 1))
                    nm = smp.tile([1, SC], F32, tag="nm", bufs=1, name=f"nm_{name}_{sc}")
                    nc.vector.tensor_scalar_mul(nm[:, :], stats[0:1, :], -1.0 / D)
                    # e2 = E[x^2] + eps - mu^2  (variance + eps)
                    e2 = smp.tile([1, SC], F32, tag="e2", bufs=1, name=f"e2_{name}_{sc}")
                    nc.vector.tensor_scalar(e2[:, :], stats[64:65, :], 1.0 / D,
                                            1e-5, op0=ALU.mult, op1=ALU.add)
                    musq = smp.tile([1, SC], F32, tag="musq", bufs=1, name=f"musq_{name}_{sc}")
                    nc.vector.tensor_mul(musq[:, :], nm[:, :], nm[:, :])
                    nc.vector.tensor_sub(e2[:, :], e2[:, :], musq[:, :])
                    # inv = rsqrt(e2) on DVE: quake seed + 2 Newton steps
                    # (no ACT Sqrt -> no table-set switch)
                    yi = smp.tile([1, SC], I32, tag="yi", bufs=1,
                                  name=f"yi_{name}_{sc}")
                    nc.vector.tensor_scalar(yi[:, :], e2[:, :].bitcast(I32),
                                            1, None, op0=ALU.logical_shift_right)
                    nc.vector.tensor_scalar(yi[:, :], yi[:, :], -1,
                                            None, op0=ALU.bitwise_xor)
                    nc.vector.tensor_scalar(yi[:, :], yi[:, :], 0x5f3759e0,
                                            None, op0=ALU.add)
                    inv = yi[:, :].bitcast(F32)
                    for it in range(2):
                        h2c = smp.tile([1, SC], F32, tag="h2c", bufs=1,
                                       name=f"h2c_{name}_{sc}_{it}")
                        nc.vector.tensor_mul(h2c[:, :], inv, inv)
                        nc.vector.tensor_mul(h2c[:, :], h2c[:, :], e2[:, :])
                        nc.vector.tensor_scalar(h2c[:, :], h2c[:, :], -0.5, 1.5,
                                                op0=ALU.mult, op1=ALU.add)
                        nc.vector.tensor_mul(inv, inv, h2c[:, :])
                    ninv = smp.tile([1, SC], F32, tag="ninv", bufs=1, name=f"ninv_{name}_{sc}")
                    nc.vector.tensor_mul(ninv[:, :], nm[:, :], inv)
                    inv16 = smp.tile([1, 2 * SC], F16, tag="inv16", bufs=1, name=f"inv16_{name}_{sc}")
                    nc.vector.tensor_copy(inv16[:, 0:SC], inv)
                    nc.vector.tensor_copy(inv16[:, SC:2 * SC], ninv[:, :])
                    bb = x2p.tile([128, 2 * SC], F16, tag="bb", bufs=1,
                                  name=f"bb_{name}_{sc}")
                    nc.gpsimd.partition_broadcast(bb[:, :], inv16[0:1, :])
                    for dc in range(NDC):
                        t1 = x2p.tile([128, SC], F16, tag="t1",
                                      name=f"t1_{name}_{sc}_{dc}", bufs=2)
                        nc.vector.tensor_mul(t1[:, :], prevT[sc][:, dc, :],
                                             bb[:, 0:SC])
                        nc.vector.tensor_add(t1[:, :], t1[:, :], bb[:, SC:2 * SC])
                        nc.vector.tensor_scalar(newT[sc][:, dc, :], t1[:, :],
                                                lng_sb[:, dc:dc + 1],
                                                lnb_sb[:, dc:dc + 1],
                                                op0=ALU.mult, op1=ALU.add)

                def attn_addin(mi):
                    def fn(sc, dc):
                        return (a_out[mi][sc][ts(dc, 128), :], None)
                    return fn

                def y_addin(sc, dc):
                    return (y_red[sc][ts(dc, 128), :], f2bT_sb[:, dc:dc + 1])

                # FFN weights resident; loaded early on scalar queue
                f1w_sb = ffp.tile([128, NDC, FS], F16, tag="f1w")
                nc.scalar.dma_start(f1w_sb[:, :, :], f1w[:, :, :])
                f2w_sb = ffp.tile([128, NFC, D], F16, tag="f2w")
                nc.scalar.dma_start(f2w_sb[:, :, :], f2w[:, :, :])

                def ffn_ar(sc):
                    nc.gpsimd.collective_compute(
                        "AllReduce", ALU.add, replica_groups=RG,
                        ins=[y_par[sc][:, :].opt()], outs=[y_red[sc][:, :].opt()],
                    )

                def ffn_chunk(h2T, sc):
                    uT = ffp.tile([128, NFC, SC], F16, tag="uT", bufs=1,
                                  name=f"uT_{sc}")
                    for fc in range(NFC):
                        pu = psa.tile([128, SC], F32, tag="ps1", name=f"pu_{fc}_{sc}")
                        for dc in range(NDC):
                            nc.tensor.matmul(pu[:, :], f1w_sb[:, dc, ts(fc, 128)],
                                             h2T[sc][:, dc, :],
                                             start=(dc == 0), stop=(dc == NDC - 1))
                        nc.scalar.activation(uT[:, fc, :], pu[:, :], AF.Relu,
                                             bias=f1b_sb[:, fc:fc + 1])
                    for dc in range(NDC):
                        py = psa.tile([128, SC], F32, tag="ps1", name=f"py_{dc}_{sc}")
                        for fc in range(NFC):
                            nc.tensor.matmul(py[:, :], f2w_sb[:, fc, ts(dc, 128)],
                                             uT[:, fc, :],
                                             start=(fc == 0), stop=(fc == NFC - 1))
                        yv = evp.tile([128, SC], F16, tag="yv", bufs=2,
                                      name=f"yv_{dc}_{sc}")
                        nc.scalar.activation(yv[:, :], py[:, :], AF.Copy)
                        nc.sync.dma_start(y_par[sc][ts(dc, 128), :], yv[:, :])

                # ---- output GEMM + softmax machinery (per s-chunk) ----
                # exp-tile buffering: tail(sc) frees eo_[sc][vc] at
                # ~AR-latency into gemm(sc+1); earlier v-chunks need a
                # second generation.
                ETC = 13
                ets = [[expp.tile([128, SC], F16, tag=f"eo_{vc}",
                                  bufs=(2 if vc < ETC else 1),
                                  name=f"eo_{sc}_{vc}")
                        for vc in range(NVC)] for sc in range(NSC)]

                def wvt_load(sc, vc):
                    wvt = owp.tile([128, NDC, 128], F16, tag="ow",
                                   name=f"ow_{sc}_{vc}")
                    nc.scalar.dma_start(wvt[:, :, :], outw[vc, :, :, :])
                    return wvt

                def out_gemm_chunk(outT, sc, wvt0, injects=()):
                    injects = dict(injects)
                    pss = psa.tile([65, SC], F32, tag="ps1", name=f"pss_{sc}")
                    wvts = {0: wvt0}
                    for vc in range(NVC):
                        if vc + 1 < NVC:
                            wvts[vc + 1] = wvt_load(sc, vc + 1)
                        pl = psa.tile([128, SC], F32, tag="po",
                                      name=f"pl_{sc}_{vc}")
                        for dc in range(NDC):
                            nc.tensor.matmul(pl[:, :], wvts[vc][:, dc, :],
                                             outT[sc][:, dc, :],
                                             start=(dc == 0), stop=(dc == NDC - 1))
                        nc.scalar.activation(ets[sc][vc][:, :], pl[:, :],
                                             AF.Exp, bias=outb_sb[:, vc:vc + 1])
                        nc.tensor.matmul(pss[0:1, :], ones_col[:, :],
                                         ets[sc][vc][:, :],
                                         start=(vc == 0), stop=(vc == NVC - 1))
                        del wvts[vc]
                        if vc in injects:
                            injects[vc]()
                    sv = smp.tile([1, SC], F32, tag="ssv", bufs=1, name=f"ssv_{sc}")
                    nc.scalar.activation(sv[:, :], pss[0:1, :], AF.Copy)
                    nc.scalar.dma_start(ss_in[sc][0:1, :], sv[:, :])
                    nc.gpsimd.collective_compute(
                        "AllReduce", ALU.add, replica_groups=RG,
                        ins=[ss_in[sc][:, :].opt()], outs=[ss_out[sc][:, :].opt()],
                    )

                def out_tail_pre(sc):
                    # reciprocal of the AllReduced exp-sums; off the PE/scalar
                    # critical queues so nothing stalls waiting for the AR.
                    rr = osc.tile([1, SC], F32, tag="rr", bufs=1, name=f"rr{sc}")
                    nc.sync.dma_start(rr[:, :], ss_out[sc][0:1, :])
                    ri = osc.tile([1, SC], F32, tag="ri", bufs=1, name=f"ri{sc}")
                    nc.vector.reciprocal_approx_fast(ri[:, :], rr[:, :])
                    ri16 = osc.tile([1, SC], F16, tag="ri16", bufs=1, name=f"ri16_{sc}")
                    nc.vector.tensor_copy(ri16[:, :], ri[:, :])
                    return ri16

                def out_tail_post(sc, ri16, last=False):
                    # emitted mid-gemm(sc+1), after the AR has landed; all
                    # muls on DVE (gpsimd stays extended-lib-only), probs
                    # writeout on sync; late v-chunks first (their
                    # single-buffered exp tiles gate gemm(sc+1)). For the
                    # final chunk the broadcast runs on the (now idle) PE
                    # and the muls read PSUM directly -- shortest chain.
                    hsl = ts(sc, SC)
                    if last:
                        recb = psa.tile([128, SC], F32, tag="po",
                                        name=f"recbp{sc}")
                        nc.tensor.matmul(recb[:, :], ones_row[:, :],
                                         ri16[0:1, :], start=True, stop=True)
                        rb_ap = recb[:, :]
                    else:
                        recb_sb = osc.tile([128, SC], F16, tag="recb", bufs=1,
                                           name=f"recb_sb{sc}")
                        nc.gpsimd.partition_broadcast(recb_sb[:, :], ri16[0:1, :])
                        rb_ap = recb_sb[:, :]
                    for vc in list(range(ETC, NVC)) + list(range(ETC)):
                        pp = ppp.tile([128, SC], F16, tag="ppv",
                                      name=f"pp_{vc}_{sc}")
                        nc.vector.tensor_mul(pp[:, :], ets[sc][vc][:, :], rb_ap)
                        nc.sync.dma_start(probsT[ts(vc, 128), hsl], pp[:, :])

                # ================= emission schedule =================
                with tc.tile_pool(name="attn", bufs=1) as attnp:
                    qkw_sbs, vw_sbs = [], []
                    for mi in range(2):
                        qs = attnp.tile([128, NDC, 256], F16, tag="qkw",
                                        name=f"qkw_sb{mi}")
                        nc.scalar.dma_start(qs[:, :, :], qkw[mi][:, :, :])
                        vs = attnp.tile([128, NDC, 130], F16, tag="vw",
                                        name=f"vw_sb{mi}")
                        nc.scalar.dma_start(vs[:, :, :], vw[mi][:, :, :])
                        qkw_sbs.append(qs)
                        vw_sbs.append(vs)

                    with tc.tile_pool(name="maskp", bufs=1) as maskp:
                        mask_sb = maskp.tile([128, 4 * SC], F16, tag="mask")
                        nc.scalar.dma_start(mask_sb[:, :], maskT[:, :])

                        V1 = attnp.tile([128, NTT, 130], F16, tag="V", name="V_sb0")
                        q1 = attnp.tile([128, NSC, SC], F16, tag="qT2", name="qT2_0")
                        k1 = attnp.tile([128, NSC, SC], F16, tag="kT2", name="kT2_0")
                        at1 = attnp.tile([128, NSC, SC], F16, tag="attnT",
                                         name="attnT0")

                        # --- MHA1 (masked) + ln1, pipelined per s-chunk.
                        # Projections hoisted ahead of the (short, latency-
                        # bound) masked chunks to keep the PE fed. ---
                        h1T = act_tiles("h1T")
                        with nc.named_scope("A1c0"):
                            mha_proj_chunk(0, 0, hT, qkw_sbs[0], vw_sbs[0],
                                           V1, q1, k1)
                            mha_proj_chunk(0, 1, hT, qkw_sbs[0], vw_sbs[0],
                                           V1, q1, k1)
                            mha_chunk(0, 0, True, at1, V1, q1, k1, mask_sb)
                        with nc.named_scope("A1c1"):
                            mha_proj_chunk(0, 2, hT, qkw_sbs[0], vw_sbs[0],
                                           V1, q1, k1)
                            mha_chunk(0, 1, True, at1, V1, q1, k1, mask_sb)
                        with nc.named_scope("A1l0"):
                            ln_chunk(hT, h1T, 0, attn_addin(0), "h1T")
                        with nc.named_scope("A1c2"):
                            mha_proj_chunk(0, 3, hT, qkw_sbs[0], vw_sbs[0],
                                           V1, q1, k1)
                            mha_chunk(0, 2, True, at1, V1, q1, k1, mask_sb)
                        with nc.named_scope("A1l1"):
                            ln_chunk(hT, h1T, 1, attn_addin(0), "h1T")
                        with nc.named_scope("A1c3"):
                            mha_chunk(0, 3, True, at1, V1, q1, k1, mask_sb)
                        with nc.named_scope("A1l2"):
                            ln_chunk(hT, h1T, 2, attn_addin(0), "h1T")

                    # --- MHA2 (unmasked) + ln2 + FFN, lag-2 pipeline.
                    # FFN AllReduces are deferred until after the last
                    # AllGather so the latency-critical gathers never queue
                    # behind them on the collective engine. ---
                    V2 = attnp.tile([128, NTT, 130], F16, tag="V", name="V_sb1")
                    q2 = attnp.tile([128, NSC, SC], F16, tag="qT2", name="qT2_1")
                    k2 = attnp.tile([128, NSC, SC], F16, tag="kT2", name="kT2_1")
                    at2 = attnp.tile([128, NSC, SC], F16, tag="attnT", name="attnT1")
                    for sc in range(3):
                        with nc.named_scope(f"A2p{sc}"):
                            mha_proj_chunk(1, sc, h1T, qkw_sbs[1], vw_sbs[1],
                                           V2, q2, k2)
                    with nc.named_scope("A1l3"):
                        ln_chunk(hT, h1T, NSC - 1, attn_addin(0), "h1T")
                    with nc.named_scope("A2p3"):
                        mha_proj_chunk(1, 3, h1T, qkw_sbs[1], vw_sbs[1],
                                       V2, q2, k2)
                    h2T = act_tiles("h2T")
                    outT = act_tiles("outT")
                    # chunks + ln2 only: the FFN work (and its AllReduces,
                    # whose inputs becoming ready would steal the collective
                    # engine from the latency-critical gathers) runs after
                    # the last AllGather is in flight.
                    for sc in range(NSC):
                        with nc.named_scope(f"A2c{sc}"):
                            mha_chunk(1, sc, False, at2, V2, q2, k2, None)
                        if sc > 0:
                            with nc.named_scope(f"A2l{sc-1}"):
                                ln_chunk(h1T, h2T, sc - 1, attn_addin(1), "h2T")
                    with nc.named_scope("A2l3"):
                        ln_chunk(h1T, h2T, NSC - 1, attn_addin(1), "h2T")
                    for sc in range(2):
                        with nc.named_scope(f"A2f{sc}"):
                            ffn_chunk(h2T, sc)
                            ffn_ar(sc)
                    with nc.named_scope("A2f2"):
                        ffn_chunk(h2T, 2)
                        ffn_ar(2)
                    with nc.named_scope("Ol0"):
                        ln_chunk(h2T, outT, 0, y_addin, "outT")
                    with nc.named_scope("A2f3"):
                        ffn_chunk(h2T, 3)
                        ffn_ar(3)
                    with nc.named_scope("Ol1"):
                        ln_chunk(h2T, outT, 1, y_addin, "outT")

                    # --- output GEMM + softmax, ln3(sc+1) and tail(sc-1)
                    #     injected mid-gemm(sc) so the PE never waits ---
                    ri16s = {}
                    for sc in range(NSC):
                        wvt0 = wvt_load(sc, 0)
                        injects = []
                        if sc > 0:
                            ri16s[sc - 1] = out_tail_pre(sc - 1)
                            injects.append(
                                (ETC, lambda p=sc - 1: out_tail_post(p, ri16s[p])))
                        if sc + 2 < NSC:
                            injects.append(
                                (20, lambda n=sc + 2: ln_chunk(
                                    h2T, outT, n, y_addin, "outT")))
                        with nc.named_scope(f"Og{sc}"):
                            out_gemm_chunk(outT, sc, wvt0, injects=injects)
                    ri16s[NSC - 1] = out_tail_pre(NSC - 1)
                    with nc.named_scope("Ot3"):
                        out_tail_post(NSC - 1, ri16s[NSC - 1], last=True)

            acts_ctx.__exit__(None, None, None)

    nc.compile()
    return nc


def _positional_encoding():
    pos = np.arange(S, dtype=np.float32)[:, None]
    i = np.arange(0, D, 2, dtype=np.float32)
    ang = (pos * np.exp((-np.log(10000.0) * i / D).astype(np.float32))).astype(np.float32)
    pe = np.zeros((S, D), np.float32)
    pe[:, 0::2] = np.sin(ang)
    pe[:, 1::2] = np.cos(ang)
    return pe


def _f16(x):
    return np.ascontiguousarray(x).astype(np.float16)


def _f32(x):
    return np.ascontiguousarray(x, dtype=np.float32)


def prepare_inputs(inp):
    """Full fp32 inputs -> per-core input maps (host-side sharding/layout)."""
    li = L - 1
    xT_full = np.ascontiguousarray(inp["x"].T)          # [V, S]
    peb = (inp["emb_b"][None, :] + _positional_encoding()).astype(np.float32)
    pebT = _f16(peb.T)                                   # [D, S] fp16

    # causal mask patterns for the 4 diagonal t-tiles of an s-chunk
    t_loc = np.arange(128)[:, None]
    s_loc = np.arange(SC)[None, :]
    maskT = np.concatenate(
        [((p * 128 + t_loc) <= s_loc).astype(np.float32) for p in range(4)], axis=1
    )
    maskT = _f16(maskT)                                  # [128, 2048]

    lngT = _f32(inp["ln_g"].reshape(NDC, 128).T)
    lnbT = _f32(inp["ln_b"].reshape(NDC, 128).T)
    f2bT = _f32(inp["ff_b2"][li].reshape(NDC, 128).T)

    in_maps = []
    for c in range(NCORES):
        m = {}
        xs = xT_full[c * VSR:(c + 1) * VSR]              # [4000, S]
        m["xT"] = _f16(np.concatenate([xs, np.zeros((VSP - VSR, S), np.float32)], 0))
        ew = inp["emb_W"][c * VSR:(c + 1) * VSR]
        m["embW"] = _f16(np.concatenate([ew, np.zeros((VSP - VSR, D), np.float32)], 0))
        m["pebT"] = pebT
        m["maskT"] = maskT
        for mi, (Wq, bq, Wk, bk, Wv, bvv) in enumerate([
            (inp["Wq1"][li], inp["bq1"][li], inp["Wk1"][li], inp["bk1"][li],
             inp["Wv1"][li], inp["bv1"][li]),
            (inp["Wq2"][li], inp["bq2"][li], inp["Wk2"][li], inp["bk2"][li],
             inp["Wv2"][li], inp["bv2"][li]),
        ]):
            h0, h1 = 2 * c, 2 * c + 1
            qk = np.concatenate([Wq[h0], Wq[h1], Wk[h0], Wk[h1]], axis=1)  # [D, 256]
            m[f"qkw{mi+1}"] = _f16(qk.reshape(NDC, 128, 256).transpose(1, 0, 2))
            m[f"bqk{mi+1}"] = _f32(np.stack(
                [np.concatenate([bq[h0], bq[h1]]),
                 np.concatenate([bk[h0], bk[h1]])], axis=1))
            vp = np.zeros((D, 130), np.float32)
            vp[:, 0:64] = Wv[h0]
            vp[:, 65:129] = Wv[h1]
            m[f"vw{mi+1}"] = _f16(vp.reshape(NDC, 128, 130).transpose(1, 0, 2))
            m[f"bv{mi+1}"] = _f32(np.concatenate([bvv[h0], bvv[h1]])[:, None])
        w1 = inp["ff_W1"][li][:, c * FS:(c + 1) * FS]    # [D, FS]
        m["f1w"] = _f16(w1.reshape(NDC, 128, FS).transpose(1, 0, 2))
        m["f1b"] = _f32(inp["ff_b1"][li][c * FS:(c + 1) * FS].reshape(NFC, 128).T)
        w2 = inp["ff_W2"][li][c * FS:(c + 1) * FS]       # [FS, D]
        m["f2w"] = _f16(w2.reshape(NFC, 128, D).transpose(1, 0, 2))
        m["f2bT"] = f2bT
        m["lngT"] = lngT
        m["lnbT"] = lnbT
        ow = inp["out_W"][:, c * VSR:(c + 1) * VSR]      # [D, 4000]
        ow = np.concatenate([ow, np.zeros((D, VSP - VSR), np.float32)], axis=1)
        m["outw"] = _f16(ow.reshape(NDC, 128, NVC, 128).transpose(2, 1, 0, 3))
        ob = np.full(VSP, -30.0, np.float32)
        ob[:VSR] = inp["out_b"][c * VSR:(c + 1) * VSR]
        m["outb"] = _f32(ob.reshape(NVC, 128).T)
        in_maps.append(m)
    return in_maps


_NC_CACHE = {}


def kernel(**inputs):
    import os
    inputs = {k: np.asarray(v, dtype=np.float32) for k, v in inputs.items()}
    if "nc" not in _NC_CACHE:
        _NC_CACHE["nc"] = build_bass()
    nc = _NC_CACHE["nc"]
    in_maps = prepare_inputs(inputs)
    trace = bool(int(os.environ.get("KB_TRACE", "0")))
    tmpdir = os.environ.get("KB_TMPDIR") or None
    res = run_bass_kernel_spmd(nc, in_maps, list(range(NCORES)), trace=trace,
                               tmpdir=tmpdir)
    LAST_RESULTS["res"] = res
    shards = [np.asarray(res.results[c]["probsT"][:VSR], dtype=np.float32)
              for c in range(NCORES)]
    return np.ascontiguousarray(np.concatenate(shards, axis=0).T)


# revision 33
# speedup vs baseline: 1.0180x; 1.0180x over previous
"""Trainium2 Bass kernel for nn_LonelyDecoder (dense transformer, 8-core TP).

v6 highlights:
 - fp16 everywhere (same PE rate as bf16, ~8x less rounding noise).
 - embW resident in SBUF; per-s-chunk activation tiles (no whole-tile
   false deps across pipeline stages).
 - One software-pipelined schedule with lag-2 stages: MHA2 chunk sc |
   ln2 chunk sc-1 | FFN chunk sc-2, and ln3(sc+1) injected mid
   output-GEMM(sc), so the LN scalar chain (DVE smalls + gpsimd
   broadcast) never sits on the PE critical path.
 - LN rsqrt computed on DVE via bitcast+Newton (no ACT Sqrt table-set
   switches); all broadcasts via gpsimd PartitionBroadcast; gpsimd kept
   extended-lib-only (library reloads cost ~8us).
 - Output GEMM per s-chunk with pipelined softmax-sum AllReduce; only
   the last chunk's AR + normalize + writeout is exposed.
"""

import numpy as np
import ml_dtypes

import concourse.bacc as bacc
import concourse.bass as bass
import concourse.mybir as mybir
import concourse.tile as tile
from concourse.bass_utils import run_bass_kernel_spmd

F32 = mybir.dt.float32
F16 = mybir.dt.float16
I32 = mybir.dt.int32
AF = mybir.ActivationFunctionType
ALU = mybir.AluOpType

S, V, D, H, DK, DFF, L = 2048, 32000, 1024, 16, 64, 4096, 4
NCORES = 8
VSR = V // NCORES          # 4000 real vocab shard
VSP = 4096                 # padded vocab shard (32 x 128)
NVC = VSP // 128           # 32 v-chunks
NDC = D // 128             # 8 d-chunks
NSC = 4                    # s-chunks of 512
SC = 512
NTT = S // 128             # 16 t-tiles
FS = DFF // NCORES         # 512 ff shard
NFC = FS // 128            # 4 ff chunks
RG = [list(range(NCORES))]

LAST_RESULTS = {}


def ts(i, n):
    return slice(i * n, (i + 1) * n)


def build_bass():
    nc = bacc.Bacc(None, target_bir_lowering=False)

    # ---- I/O ----
    xT = nc.dram_tensor("xT", [VSP, S], F16, kind="ExternalInput")
    embW = nc.dram_tensor("embW", [VSP, D], F16, kind="ExternalInput")
    pebT = nc.dram_tensor("pebT", [D, S], F16, kind="ExternalInput")
    qkw = [nc.dram_tensor(f"qkw{m}", [128, NDC, 256], F16, kind="ExternalInput") for m in (1, 2)]
    bqk = [nc.dram_tensor(f"bqk{m}", [128, 2], F32, kind="ExternalInput") for m in (1, 2)]
    vw = [nc.dram_tensor(f"vw{m}", [128, NDC, 130], F16, kind="ExternalInput") for m in (1, 2)]
    bv = [nc.dram_tensor(f"bv{m}", [128, 1], F32, kind="ExternalInput") for m in (1, 2)]
    maskT = nc.dram_tensor("maskT", [128, 4 * SC], F16, kind="ExternalInput")
    f1w = nc.dram_tensor("f1w", [128, NDC, FS], F16, kind="ExternalInput")
    f1b = nc.dram_tensor("f1b", [128, NFC], F32, kind="ExternalInput")
    f2w = nc.dram_tensor("f2w", [128, NFC, D], F16, kind="ExternalInput")
    f2bT = nc.dram_tensor("f2bT", [128, NDC], F32, kind="ExternalInput")
    lngT = nc.dram_tensor("lngT", [128, NDC], F32, kind="ExternalInput")
    lnbT = nc.dram_tensor("lnbT", [128, NDC], F32, kind="ExternalInput")
    outw = nc.dram_tensor("outw", [NVC, 128, NDC, 128], F16, kind="ExternalInput")
    outb = nc.dram_tensor("outb", [128, NVC], F32, kind="ExternalInput")
    probsT = nc.dram_tensor("probsT", [VSP, S], F16, kind="ExternalOutput")

    with tile.TileContext(nc) as tc:
        with tc.tile_pool(name="dram", bufs=1, space="DRAM") as dram, \
             tc.tile_pool(name="const", bufs=1) as const:

            # internal DRAM (collective bounce buffers)
            h_par = [dram.tile([D, SC], F16, tag=f"hp{sc}", name=f"h_par{sc}")
                     for sc in range(NSC)]
            h_red = [dram.tile([D, SC], F16, tag=f"hr{sc}", addr_space="Shared",
                               name=f"h_red{sc}") for sc in range(NSC)]
            a_in = [[dram.tile([128, SC], F16, tag=f"a{m}i{sc}", name=f"a{m}_in{sc}")
                     for sc in range(NSC)] for m in (0, 1)]
            a_out = [[dram.tile([D, SC], F16, tag=f"a{m}o{sc}", addr_space="Shared",
                                name=f"a{m}_out{sc}") for sc in range(NSC)]
                     for m in (0, 1)]
            y_par = [dram.tile([D, SC], F16, tag=f"yp{sc}", name=f"y_par{sc}")
                     for sc in range(NSC)]
            y_red = [dram.tile([D, SC], F16, tag=f"yr{sc}", addr_space="Shared",
                               name=f"y_red{sc}") for sc in range(NSC)]
            ss_in = [dram.tile([1, SC], F32, tag=f"ssi{sc}", name=f"ss_in{sc}")
                     for sc in range(NSC)]
            ss_out = [dram.tile([1, SC], F32, tag=f"sso{sc}", addr_space="Shared",
                                name=f"ss_out{sc}") for sc in range(NSC)]

            # constants (scalar DMA queue; sync stays free for GEMM tiles)
            ones_col = const.tile([128, 1], F16, tag="c1")
            nc.vector.memset(ones_col[:, :], 1.0)
            ones_row = const.tile([1, 128], F16, tag="c3")
            nc.vector.memset(ones_row[:, :], 1.0)
            bqk_sb = [const.tile([128, 2], F32, tag=f"bqk{m}", name=f"bqk_sb{m}") for m in range(2)]
            bv_sb = [const.tile([128, 1], F32, tag=f"bv{m}", name=f"bv_sb{m}") for m in range(2)]
            for m in range(2):
                nc.scalar.dma_start(bqk_sb[m][:, :], bqk[m][:, :])
                nc.scalar.dma_start(bv_sb[m][:, :], bv[m][:, :])
            f1b_sb = const.tile([128, NFC], F32, tag="f1b")
            nc.scalar.dma_start(f1b_sb[:, :], f1b[:, :])
            f2bT_sb = const.tile([128, NDC], F32, tag="f2bT")
            nc.scalar.dma_start(f2bT_sb[:, :], f2bT[:, :])
            lng_sb = const.tile([128, NDC], F32, tag="lng")
            nc.scalar.dma_start(lng_sb[:, :], lngT[:, :])
            lnb_sb = const.tile([128, NDC], F32, tag="lnb")
            nc.scalar.dma_start(lnb_sb[:, :], lnbT[:, :])
            outb_sb = const.tile([128, NVC], F32, tag="outb")
            nc.scalar.dma_start(outb_sb[:, :], outb[:, :])

            # per-s-chunk activation tiles: tag per sc so cross-chunk
            # consumers never pick up whole-tile false dependencies.
            acts_ctx = tc.tile_pool(name="acts", bufs=2)
            acts = acts_ctx.__enter__()

            def act_tiles(name):
                return [acts.tile([128, NDC, SC], F16, tag=f"act{sc}",
                                  name=f"{name}{sc}") for sc in range(NSC)]

            hT = act_tiles("hT")

            # ---------- phase E: embedding GEMM (chunked AllReduce) ----------
            with tc.tile_pool(name="embw", bufs=1) as embp, \
                 tc.tile_pool(name="xt", bufs=6) as xtp, \
                 tc.tile_pool(name="peb", bufs=1) as pebp, \
                 tc.tile_pool(name="ps_e", bufs=1, space="PSUM") as pse, \
                 tc.tile_pool(name="ev_e", bufs=3) as evp, \
                 tc.tile_pool(name="addin_e", bufs=3) as adpe:
                # embW resident (64KB/partition), loaded once on scalar queue
                ew_sb = embp.tile([128, NVC, D], F16, tag="ew")
                peb_sb = pebp.tile([128, NDC, S], F16, tag="peb")
                for dc in range(NDC):
                    nc.gpsimd.dma_start(peb_sb[:, dc, :], pebT[ts(dc, 128), :])

                def ht_prep(psc):
                    for dc in range(NDC):
                        hr = adpe.tile([128, SC], F16, tag="addin",
                                       name=f"hr_{psc}_{dc}")
                        nc.scalar.dma_start(hr[:, :], h_red[psc][ts(dc, 128), :])
                        nc.vector.tensor_add(hT[psc][:, dc, :], hr[:, :],
                                             peb_sb[:, dc, ts(psc, SC)])

                for sc in range(NSC):
                    with nc.named_scope(f"E{sc}"):
                        pes = [pse.tile([128, SC], F32, tag=f"pe{dc}",
                                        name=f"pe_{sc}_{dc}")
                               for dc in range(NDC)]
                        for kc in range(NVC):
                            xt = xtp.tile([128, SC], F16, tag="xt")
                            nc.sync.dma_start(xt[:, :], xT[ts(kc, 128), ts(sc, SC)])
                            if sc == 0:
                                nc.scalar.dma_start(ew_sb[:, kc, :],
                                                    embW[ts(kc, 128), :])
                            for dc in range(NDC):
                                nc.tensor.matmul(
                                    pes[dc][:, :],
                                    ew_sb[:, kc, ts(dc, 128)],
                                    xt[:, :],
                                    start=(kc == 0),
                                    stop=(kc == NVC - 1),
                                )
                        for dc in range(NDC):
                            hv = evp.tile([128, SC], F16, tag="ev")
                            nc.scalar.activation(hv[:, :], pes[dc][:, :], AF.Copy)
                            nc.sync.dma_start(h_par[sc][ts(dc, 128), :], hv[:, :])
                        nc.gpsimd.collective_compute(
                            "AllReduce", ALU.add, replica_groups=RG,
                            ins=[h_par[sc][:, :].opt()],
                            outs=[h_red[sc][:, :].opt()],
                        )
                        if sc > 0:
                            ht_prep(sc - 1)
                ht_prep(NSC - 1)

            # ======== pipelined layer ========
            # unified PSUM pool (8 banks): ps1(2) + po(2) + pg(2x2=4)
            with tc.tile_pool(name="addin", bufs=3) as adp, \
                 tc.tile_pool(name="x2p", bufs=2) as x2p, \
                 tc.tile_pool(name="ev_a", bufs=2) as evp, \
                 tc.tile_pool(name="small_a", bufs=2) as smp, \
                 tc.tile_pool(name="osc", bufs=2) as osc, \
                 tc.tile_pool(name="pp", bufs=3) as ppp, \
                 tc.tile_pool(name="outwp", bufs=2) as owp, \
                 tc.tile_pool(name="ffw", bufs=1) as ffp, \
                 tc.tile_pool(name="exp", bufs=2) as expp, \
                 tc.tile_pool(name="ps_a", bufs=2, space="PSUM") as psa:

                def mha_proj_chunk(mi, sc, actT, qkw_sb, vw_sb, V_sb, qT2, kT2):
                    """QKV projections for weight-set mi, one s-chunk."""
                    for tt in range(4 * sc, 4 * sc + 4):
                        pv = psa.tile([128, SC], F32, tag="ps1", name=f"pv{mi}_{tt}")
                        for dc in range(NDC):
                            nc.tensor.matmul(
                                pv[:, 0:130], actT[sc][:, dc, ts(tt % 4, 128)],
                                vw_sb[:, dc, :],
                                start=(dc == 0), stop=(dc == NDC - 1),
                            )
                        nc.scalar.activation(V_sb[:, tt, :], pv[:, 0:130], AF.Copy)
                    for wi, dst in ((0, qT2), (1, kT2)):
                        pq = psa.tile([128, SC], F32, tag="ps1", name=f"pq{mi}_{wi}_{sc}")
                        for dc in range(NDC):
                            nc.tensor.matmul(
                                pq[:, :],
                                qkw_sb[:, dc, ts(wi, 128)],
                                actT[sc][:, dc, :],
                                start=(dc == 0), stop=(dc == NDC - 1),
                            )
                        nc.scalar.activation(
                            dst[:, sc, :], pq[:, :], AF.Identity,
                            bias=bqk_sb[mi][:, wi:wi + 1],
                        )
                    nc.vector.memset(V_sb[:, ts(sc, 4), 64:65], 1.0)
                    nc.vector.memset(V_sb[:, ts(sc, 4), 129:130], 1.0)

                def mha_chunk(mi, sc, masked, attnT, V_sb, qT2, kT2, mask_sb):
                    """scores+AV+normalize for one s-chunk, both heads, then
                    a_in DMA + AllGather."""
                    for h in range(2):
                        po = psa.tile([128, SC], F32, tag="po", name=f"po{mi}_{h}_{sc}")
                        tts = list(range(4 * (sc + 1))) if masked else list(range(NTT))
                        pairs = [tts[i:i + 2] for i in range(0, len(tts), 2)]
                        for pi, pr in enumerate(pairs):
                            pg = psa.tile([128, 2 * SC], F32, tag="pg",
                                          name=f"pg{mi}_{h}_{sc}_{pi}")
                            for j, tt in enumerate(pr):
                                nc.tensor.matmul(
                                    pg[:, ts(j, SC)],
                                    kT2[ts(h, 64), tt // 4, ts(tt % 4, 128)],
                                    qT2[ts(h, 64), sc, :],
                                    start=True, stop=True,
                                )
                            et = evp.tile([128, 2 * SC], F16, tag="exp")
                            nc.scalar.activation(et[:, :], pg[:, :], AF.Exp,
                                                 scale=1.0 / D)
                            if masked and pr[0] >= 4 * sc:
                                mo = (pr[0] - 4 * sc) * SC
                                nc.vector.tensor_mul(
                                    et[:, :], et[:, :],
                                    mask_sb[:, mo:mo + 2 * SC],
                                )
                            for j, tt in enumerate(pr):
                                nc.tensor.matmul(
                                    po[0:65, :],
                                    V_sb[:, tt, ts(h, 65)],
                                    et[:, ts(j, SC)],
                                    start=(pi == 0 and j == 0),
                                    stop=(pi == len(pairs) - 1 and j == len(pr) - 1),
                                )
                        oo = smp.tile([64, SC], F16, tag="oo", bufs=1, name=f"oo{mi}_{h}_{sc}")
                        nc.scalar.activation(oo[:, :], po[0:64, :], AF.Copy)
                        s0 = smp.tile([1, SC], F32, tag="s0", bufs=1, name=f"s0{mi}_{h}_{sc}")
                        nc.scalar.activation(s0[:, :], po[64:65, :], AF.Copy)
                        rec = smp.tile([1, SC], F32, tag="rec", bufs=1, name=f"rec{mi}_{h}_{sc}")
                        nc.vector.reciprocal_approx_fast(rec[:, :], s0[:, :])
                        rec16 = smp.tile([1, SC], F16, tag="rec16", bufs=1, name=f"rec16_{mi}_{h}_{sc}")
                        nc.vector.tensor_copy(rec16[:, :], rec[:, :])
                        rb = smp.tile([64, SC], F16, tag="rb", bufs=1, name=f"rb{mi}_{h}_{sc}")
                        nc.gpsimd.partition_broadcast(rb[:, :], rec16[0:1, :])
                        nc.vector.tensor_mul(oo[:, :], oo[:, :], rb[:, :])
                        nc.scalar.activation(
                            attnT[ts(h, 64), sc, :], oo[:, :], AF.Identity,
                            bias=bv_sb[mi][ts(h, 64), :],
                        )
                    nc.sync.dma_start(a_in[mi][sc][:, :], attnT[:, sc, :])
                    nc.gpsimd.collective_compute(
                        "AllGather", ALU.bypass, replica_groups=RG,
                        ins=[a_in[mi][sc][:, :].opt()],
                        outs=[a_out[mi][sc][:, :].opt()],
                    )

                # residual + layernorm over the feature dim for ONE s-chunk.
                # prevT/newT are lists of per-sc tiles [128, NDC, SC].
                def ln_chunk(prevT, newT, sc, addin_fn, name):
                    stats = psa.tile([65, SC], F32, tag="ps1",
                                     name=f"st_{name}_{sc}")
                    for dc in range(NDC):
                        src_ap, xbias = addin_fn(sc, dc)
                        ad = adp.tile([128, SC], F16, tag="addin",
                                      name=f"ad_{name}_{sc}_{dc}")
                        nc.scalar.dma_start(ad[:, :], src_ap)
                        if xbias is not None:
                            nc.vector.scalar_tensor_tensor(
                                prevT[sc][:, dc, :], ad[:, :], xbias,
                                prevT[sc][:, dc, :], op0=ALU.add, op1=ALU.add)
                        else:
                            nc.vector.tensor_add(prevT[sc][:, dc, :],
                                                 prevT[sc][:, dc, :], ad[:, :])
                        x2 = x2p.tile([128, SC], F16, tag="x2",
                                      name=f"x2_{name}_{sc}_{dc}")
                        nc.vector.tensor_mul(x2[:, :], prevT[sc][:, dc, :],
                                             prevT[sc][:, dc, :])
                        nc.tensor.matmul(stats[0:1, :], ones_col[:, :],
                                         prevT[sc][:, dc, :],
                                         start=(dc == 0), stop=(dc == NDC - 1))
                        nc.tensor.matmul(stats[64:65, :], ones_col[:, :],
                                         x2[:, :],
                                         start=(dc == 0), stop=(dc == NDC - 1))
                    nm = smp.tile([1, SC], F32, tag="nm", bufs=1, name=f"nm_{name}_{sc}")
                    nc.vector.tensor_scalar_mul(nm[:, :], stats[0:1, :], -1.0 / D)
                    # e2 = E[x^2] + eps - mu^2  (variance + eps)
                    e2 = smp.tile([1, SC], F32, tag="e2", bufs=1, name=f"e2_{name}_{sc}")
                    nc.vector.tensor_scalar(e2[:, :], stats[64:65, :], 1.0 / D,
                                            1e-5, op0=ALU.mult, op1=ALU.add)
                    musq = smp.tile([1, SC], F32, tag="musq", bufs=1, name=f"musq_{name}_{sc}")
                    nc.vector.tensor_mul(musq[:, :], nm[:, :], nm[:, :])
                    nc.vector.tensor_sub(e2[:, :], e2[:, :], musq[:, :])
                    # inv = rsqrt(e2) on DVE: quake seed + 2 Newton steps
                    # (no ACT Sqrt -> no table-set switch)
                    yi = smp.tile([1, SC], I32, tag="yi", bufs=1,
                                  name=f"yi_{name}_{sc}")
                    nc.vector.tensor_scalar(yi[:, :], e2[:, :].bitcast(I32),
                                            1, None, op0=ALU.logical_shift_right)
                    nc.vector.tensor_scalar(yi[:, :], yi[:, :], -1,
                                            None, op0=ALU.bitwise_xor)
                    nc.vector.tensor_scalar(yi[:, :], yi[:, :], 0x5f3759e0,
                                            None, op0=ALU.add)
                    inv = yi[:, :].bitcast(F32)
                    for it in range(2):
                        h2c = smp.tile([1, SC], F32, tag="h2c", bufs=1,
                                       name=f"h2c_{name}_{sc}_{it}")
                        nc.vector.tensor_mul(h2c[:, :], inv, inv)
                        nc.vector.tensor_mul(h2c[:, :], h2c[:, :], e2[:, :])
                        nc.vector.tensor_scalar(h2c[:, :], h2c[:, :], -0.5, 1.5,
                                                op0=ALU.mult, op1=ALU.add)
                        nc.vector.tensor_mul(inv, inv, h2c[:, :])
                    ninv = smp.tile([1, SC], F32, tag="ninv", bufs=1, name=f"ninv_{name}_{sc}")
                    nc.vector.tensor_mul(ninv[:, :], nm[:, :], inv)
                    inv16 = smp.tile([1, 2 * SC], F16, tag="inv16", bufs=1, name=f"inv16_{name}_{sc}")
                    nc.vector.tensor_copy(inv16[:, 0:SC], inv)
                    nc.vector.tensor_copy(inv16[:, SC:2 * SC], ninv[:, :])
                    bb = x2p.tile([128, 2 * SC], F16, tag="bb", bufs=1,
                                  name=f"bb_{name}_{sc}")
                    nc.gpsimd.partition_broadcast(bb[:, :], inv16[0:1, :])
                    for dc in range(NDC):
                        t1 = x2p.tile([128, SC], F16, tag="t1",
                                      name=f"t1_{name}_{sc}_{dc}", bufs=2)
                        nc.vector.tensor_mul(t1[:, :], prevT[sc][:, dc, :],
                                             bb[:, 0:SC])
                        nc.vector.tensor_add(t1[:, :], t1[:, :], bb[:, SC:2 * SC])
                        nc.vector.tensor_scalar(newT[sc][:, dc, :], t1[:, :],
                                                lng_sb[:, dc:dc + 1],
                                                lnb_sb[:, dc:dc + 1],
                                                op0=ALU.mult, op1=ALU.add)

                def attn_addin(mi):
                    def fn(sc, dc):
                        return (a_out[mi][sc][ts(dc, 128), :], None)
                    return fn

                def y_addin(sc, dc):
                    return (y_red[sc][ts(dc, 128), :], f2bT_sb[:, dc:dc + 1])

                # FFN weights resident; loaded early on scalar queue
                f1w_sb = ffp.tile([128, NDC, FS], F16, tag="f1w")
                nc.scalar.dma_start(f1w_sb[:, :, :], f1w[:, :, :])
                f2w_sb = ffp.tile([128, NFC, D], F16, tag="f2w")
                nc.scalar.dma_start(f2w_sb[:, :, :], f2w[:, :, :])

                def ffn_ar(sc):
                    nc.gpsimd.collective_compute(
                        "AllReduce", ALU.add, replica_groups=RG,
                        ins=[y_par[sc][:, :].opt()], outs=[y_red[sc][:, :].opt()],
                    )

                def ffn_chunk(h2T, sc):
                    uT = ffp.tile([128, NFC, SC], F16, tag="uT", bufs=1,
                                  name=f"uT_{sc}")
                    for fc in range(NFC):
                        pu = psa.tile([128, SC], F32, tag="ps1", name=f"pu_{fc}_{sc}")
                        for dc in range(NDC):
                            nc.tensor.matmul(pu[:, :], f1w_sb[:, dc, ts(fc, 128)],
                                             h2T[sc][:, dc, :],
                                             start=(dc == 0), stop=(dc == NDC - 1))
                        nc.scalar.activation(uT[:, fc, :], pu[:, :], AF.Relu,
                                             bias=f1b_sb[:, fc:fc + 1])
                    for dc in range(NDC):
                        py = psa.tile([128, SC], F32, tag="ps1", name=f"py_{dc}_{sc}")
                        for fc in range(NFC):
                            nc.tensor.matmul(py[:, :], f2w_sb[:, fc, ts(dc, 128)],
                                             uT[:, fc, :],
                                             start=(fc == 0), stop=(fc == NFC - 1))
                        yv = evp.tile([128, SC], F16, tag="yv", bufs=2,
                                      name=f"yv_{dc}_{sc}")
                        nc.scalar.activation(yv[:, :], py[:, :], AF.Copy)
                        nc.sync.dma_start(y_par[sc][ts(dc, 128), :], yv[:, :])

                # ---- output GEMM + softmax machinery (per s-chunk) ----
                # exp-tile buffering: tail(sc) frees eo_[sc][vc] at
                # ~AR-latency into gemm(sc+1); earlier v-chunks need a
                # second generation.
                ETC = 13
                ets = [[expp.tile([128, SC], F16, tag=f"eo_{vc}",
                                  bufs=(2 if vc < ETC else 1),
                                  name=f"eo_{sc}_{vc}")
                        for vc in range(NVC)] for sc in range(NSC)]

                def wvt_load(sc, vc):
                    wvt = owp.tile([128, NDC, 128], F16, tag="ow",
                                   name=f"ow_{sc}_{vc}")
                    nc.scalar.dma_start(wvt[:, :, :], outw[vc, :, :, :])
                    return wvt

                def out_gemm_chunk(outT, sc, wvt0, injects=()):
                    injects = dict(injects)
                    pss = psa.tile([65, SC], F32, tag="ps1", name=f"pss_{sc}")
                    wvts = {0: wvt0}
                    for vc in range(NVC):
                        if vc + 1 < NVC:
                            wvts[vc + 1] = wvt_load(sc, vc + 1)
                        pl = psa.tile([128, SC], F32, tag="po",
                                      name=f"pl_{sc}_{vc}")
                        for dc in range(NDC):
                            nc.tensor.matmul(pl[:, :], wvts[vc][:, dc, :],
                                             outT[sc][:, dc, :],
                                             start=(dc == 0), stop=(dc == NDC - 1))
                        nc.scalar.activation(ets[sc][vc][:, :], pl[:, :],
                                             AF.Exp, bias=outb_sb[:, vc:vc + 1])
                        nc.tensor.matmul(pss[0:1, :], ones_col[:, :],
                                         ets[sc][vc][:, :],
                                         start=(vc == 0), stop=(vc == NVC - 1))
                        del wvts[vc]
                        if vc in injects:
                            injects[vc]()
                    sv = smp.tile([1, SC], F32, tag="ssv", bufs=1, name=f"ssv_{sc}")
                    nc.scalar.activation(sv[:, :], pss[0:1, :], AF.Copy)
                    nc.scalar.dma_start(ss_in[sc][0:1, :], sv[:, :])
                    nc.gpsimd.collective_compute(
                        "AllReduce", ALU.add, replica_groups=RG,
                        ins=[ss_in[sc][:, :].opt()], outs=[ss_out[sc][:, :].opt()],
                    )

                def out_tail_pre(sc):
                    # reciprocal of the AllReduced exp-sums; off the PE/scalar
                    # critical queues so nothing stalls waiting for the AR.
                    rr = osc.tile([1, SC], F32, tag="rr", bufs=1, name=f"rr{sc}")
                    nc.sync.dma_start(rr[:, :], ss_out[sc][0:1, :])
                    ri = osc.tile([1, SC], F32, tag="ri", bufs=1, name=f"ri{sc}")
                    nc.vector.reciprocal_approx_fast(ri[:, :], rr[:, :])
                    ri16 = osc.tile([1, SC], F16, tag="ri16", bufs=1, name=f"ri16_{sc}")
                    nc.vector.tensor_copy(ri16[:, :], ri[:, :])
                    return ri16

                def out_tail_post(sc, ri16, last=False):
                    # emitted mid-gemm(sc+1), after the AR has landed; all
                    # muls on DVE (gpsimd stays extended-lib-only), probs
                    # writeout on sync; late v-chunks first (their
                    # single-buffered exp tiles gate gemm(sc+1)). For the
                    # final chunk the broadcast runs on the (now idle) PE
                    # and the muls read PSUM directly -- shortest chain.
                    hsl = ts(sc, SC)
                    if last:
                        recb = psa.tile([128, SC], F32, tag="po",
                                        name=f"recbp{sc}")
                        nc.tensor.matmul(recb[:, :], ones_row[:, :],
                                         ri16[0:1, :], start=True, stop=True)
                        rb_ap = recb[:, :]
                    else:
                        recb_sb = osc.tile([128, SC], F16, tag="recb", bufs=1,
                                           name=f"recb_sb{sc}")
                        nc.gpsimd.partition_broadcast(recb_sb[:, :], ri16[0:1, :])
                        rb_ap = recb_sb[:, :]
                    for vc in list(range(ETC, NVC)) + list(range(ETC)):
                        pp = ppp.tile([128, SC], F16, tag="ppv",
                                      name=f"pp_{vc}_{sc}")
                        nc.vector.tensor_mul(pp[:, :], ets[sc][vc][:, :], rb_ap)
                        nc.sync.dma_start(probsT[ts(vc, 128), hsl], pp[:, :])

                # ================= emission schedule =================
                with tc.tile_pool(name="attn", bufs=1) as attnp:
                    qkw_sbs, vw_sbs = [], []
                    for mi in range(2):
                        qs = attnp.tile([128, NDC, 256], F16, tag="qkw",
                                        name=f"qkw_sb{mi}")
                        nc.scalar.dma_start(qs[:, :, :], qkw[mi][:, :, :])
                        vs = attnp.tile([128, NDC, 130], F16, tag="vw",
                                        name=f"vw_sb{mi}")
                        nc.scalar.dma_start(vs[:, :, :], vw[mi][:, :, :])
                        qkw_sbs.append(qs)
                        vw_sbs.append(vs)

                    with tc.tile_pool(name="maskp", bufs=1) as maskp:
                        mask_sb = maskp.tile([128, 4 * SC], F16, tag="mask")
                        nc.scalar.dma_start(mask_sb[:, :], maskT[:, :])

                        V1 = attnp.tile([128, NTT, 130], F16, tag="V", name="V_sb0")
                        q1 = attnp.tile([128, NSC, SC], F16, tag="qT2", name="qT2_0")
                        k1 = attnp.tile([128, NSC, SC], F16, tag="kT2", name="kT2_0")
                        at1 = attnp.tile([128, NSC, SC], F16, tag="attnT",
                                         name="attnT0")

                        # --- MHA1 (masked) + ln1, pipelined per s-chunk.
                        # Projections hoisted ahead of the (short, latency-
                        # bound) masked chunks to keep the PE fed. ---
                        h1T = act_tiles("h1T")
                        with nc.named_scope("A1c0"):
                            mha_proj_chunk(0, 0, hT, qkw_sbs[0], vw_sbs[0],
                                           V1, q1, k1)
                            mha_proj_chunk(0, 1, hT, qkw_sbs[0], vw_sbs[0],
                                           V1, q1, k1)
                            mha_chunk(0, 0, True, at1, V1, q1, k1, mask_sb)
                        with nc.named_scope("A1c1"):
                            mha_proj_chunk(0, 2, hT, qkw_sbs[0], vw_sbs[0],
                                           V1, q1, k1)
                            mha_chunk(0, 1, True, at1, V1, q1, k1, mask_sb)
                        with nc.named_scope("A1l0"):
                            ln_chunk(hT, h1T, 0, attn_addin(0), "h1T")
                        with nc.named_scope("A1c2"):
                            mha_proj_chunk(0, 3, hT, qkw_sbs[0], vw_sbs[0],
                                           V1, q1, k1)
                            mha_chunk(0, 2, True, at1, V1, q1, k1, mask_sb)
                        with nc.named_scope("A1l1"):
                            ln_chunk(hT, h1T, 1, attn_addin(0), "h1T")
                        with nc.named_scope("A1c3"):
                            mha_chunk(0, 3, True, at1, V1, q1, k1, mask_sb)
                        with nc.named_scope("A1l2"):
                            ln_chunk(hT, h1T, 2, attn_addin(0), "h1T")

                    # --- MHA2 (unmasked) + ln2 + FFN, lag-2 pipeline.
                    # FFN AllReduces are deferred until after the last
                    # AllGather so the latency-critical gathers never queue
                    # behind them on the collective engine. ---
                    V2 = attnp.tile([128, NTT, 130], F16, tag="V", name="V_sb1")
                    q2 = attnp.tile([128, NSC, SC], F16, tag="qT2", name="qT2_1")
                    k2 = attnp.tile([128, NSC, SC], F16, tag="kT2", name="kT2_1")
                    at2 = attnp.tile([128, NSC, SC], F16, tag="attnT", name="attnT1")
                    for sc in range(3):
                        with nc.named_scope(f"A2p{sc}"):
                            mha_proj_chunk(1, sc, h1T, qkw_sbs[1], vw_sbs[1],
                                           V2, q2, k2)
                    with nc.named_scope("A1l3"):
                        ln_chunk(hT, h1T, NSC - 1, attn_addin(0), "h1T")
                    with nc.named_scope("A2p3"):
                        mha_proj_chunk(1, 3, h1T, qkw_sbs[1], vw_sbs[1],
                                       V2, q2, k2)
                    h2T = act_tiles("h2T")
                    outT = act_tiles("outT")
                    # chunks + ln2 only: the FFN work (and its AllReduces,
                    # whose inputs becoming ready would steal the collective
                    # engine from the latency-critical gathers) runs after
                    # the last AllGather is in flight.
                    for sc in range(NSC):
                        with nc.named_scope(f"A2c{sc}"):
                            mha_chunk(1, sc, False, at2, V2, q2, k2, None)
                        if sc > 0:
                            with nc.named_scope(f"A2l{sc-1}"):
                                ln_chunk(h1T, h2T, sc - 1, attn_addin(1), "h2T")
                        if sc == 3:
                            with nc.named_scope("A2f0"):
                                ffn_chunk(h2T, 0)
                                ffn_ar(0)
                    with nc.named_scope("A2l3"):
                        ln_chunk(h1T, h2T, NSC - 1, attn_addin(1), "h2T")
                    with nc.named_scope("A2f1"):
                        ffn_chunk(h2T, 1)
                        ffn_ar(1)
                    with nc.named_scope("A2f2"):
                        ffn_chunk(h2T, 2)
                        ffn_ar(2)
                    with nc.named_scope("Ol0"):
                        ln_chunk(h2T, outT, 0, y_addin, "outT")
                    with nc.named_scope("A2f3"):
                        ffn_chunk(h2T, 3)
                        ffn_ar(3)
                    with nc.named_scope("Ol1"):
                        ln_chunk(h2T, outT, 1, y_addin, "outT")

                    # --- output GEMM + softmax, ln3(sc+1) and tail(sc-1)
                    #     injected mid-gemm(sc) so the PE never waits ---
                    ri16s = {}
                    for sc in range(NSC):
                        wvt0 = wvt_load(sc, 0)
                        injects = []
                        if sc > 0:
                            ri16s[sc - 1] = out_tail_pre(sc - 1)
                            injects.append(
                                (ETC, lambda p=sc - 1: out_tail_post(p, ri16s[p])))
                        if sc + 2 < NSC:
                            injects.append(
                                (20, lambda n=sc + 2: ln_chunk(
                                    h2T, outT, n, y_addin, "outT")))
                        with nc.named_scope(f"Og{sc}"):
                            out_gemm_chunk(outT, sc, wvt0, injects=injects)
                    ri16s[NSC - 1] = out_tail_pre(NSC - 1)
                    with nc.named_scope("Ot3"):
                        out_tail_post(NSC - 1, ri16s[NSC - 1], last=True)

            acts_ctx.__exit__(None, None, None)

    nc.compile()
    return nc


def _positional_encoding():
    pos = np.arange(S, dtype=np.float32)[:, None]
    i = np.arange(0, D, 2, dtype=np.float32)
    ang = (pos * np.exp((-np.log(10000.0) * i / D).astype(np.float32))).astype(np.float32)
    pe = np.zeros((S, D), np.float32)
    pe[:, 0::2] = np.sin(ang)
    pe[:, 1::2] = np.cos(ang)
    return pe


def _f16(x):
    return np.ascontiguousarray(x).astype(np.float16)


def _f32(x):
    return np.ascontiguousarray(x, dtype=np.float32)


def prepare_inputs(inp):
    """Full fp32 inputs -> per-core input maps (host-side sharding/layout)."""
    li = L - 1
    xT_full = np.ascontiguousarray(inp["x"].T)          # [V, S]
    peb = (inp["emb_b"][None, :] + _positional_encoding()).astype(np.float32)
    pebT = _f16(peb.T)                                   # [D, S] fp16

    # causal mask patterns for the 4 diagonal t-tiles of an s-chunk
    t_loc = np.arange(128)[:, None]
    s_loc = np.arange(SC)[None, :]
    maskT = np.concatenate(
        [((p * 128 + t_loc) <= s_loc).astype(np.float32) for p in range(4)], axis=1
    )
    maskT = _f16(maskT)                                  # [128, 2048]

    lngT = _f32(inp["ln_g"].reshape(NDC, 128).T)
    lnbT = _f32(inp["ln_b"].reshape(NDC, 128).T)
    f2bT = _f32(inp["ff_b2"][li].reshape(NDC, 128).T)

    in_maps = []
    for c in range(NCORES):
        m = {}
        xs = xT_full[c * VSR:(c + 1) * VSR]              # [4000, S]
        m["xT"] = _f16(np.concatenate([xs, np.zeros((VSP - VSR, S), np.float32)], 0))
        ew = inp["emb_W"][c * VSR:(c + 1) * VSR]
        m["embW"] = _f16(np.concatenate([ew, np.zeros((VSP - VSR, D), np.float32)], 0))
        m["pebT"] = pebT
        m["maskT"] = maskT
        for mi, (Wq, bq, Wk, bk, Wv, bvv) in enumerate([
            (inp["Wq1"][li], inp["bq1"][li], inp["Wk1"][li], inp["bk1"][li],
             inp["Wv1"][li], inp["bv1"][li]),
            (inp["Wq2"][li], inp["bq2"][li], inp["Wk2"][li], inp["bk2"][li],
             inp["Wv2"][li], inp["bv2"][li]),
        ]):
            h0, h1 = 2 * c, 2 * c + 1
            qk = np.concatenate([Wq[h0], Wq[h1], Wk[h0], Wk[h1]], axis=1)  # [D, 256]
            m[f"qkw{mi+1}"] = _f16(qk.reshape(NDC, 128, 256).transpose(1, 0, 2))
            m[f"bqk{mi+1}"] = _f32(np.stack(
                [np.concatenate([bq[h0], bq[h1]]),
                 np.concatenate([bk[h0], bk[h1]])], axis=1))
            vp = np.zeros((D, 130), np.float32)
            vp[:, 0:64] = Wv[h0]
            vp[:, 65:129] = Wv[h1]
            m[f"vw{mi+1}"] = _f16(vp.reshape(NDC, 128, 130).transpose(1, 0, 2))
            m[f"bv{mi+1}"] = _f32(np.concatenate([bvv[h0], bvv[h1]])[:, None])
        w1 = inp["ff_W1"][li][:, c * FS:(c + 1) * FS]    # [D, FS]
        m["f1w"] = _f16(w1.reshape(NDC, 128, FS).transpose(1, 0, 2))
        m["f1b"] = _f32(inp["ff_b1"][li][c * FS:(c + 1) * FS].reshape(NFC, 128).T)
        w2 = inp["ff_W2"][li][c * FS:(c + 1) * FS]       # [FS, D]
        m["f2w"] = _f16(w2.reshape(NFC, 128, D).transpose(1, 0, 2))
        m["f2bT"] = f2bT
        m["lngT"] = lngT
        m["lnbT"] = lnbT
        ow = inp["out_W"][:, c * VSR:(c + 1) * VSR]      # [D, 4000]
        ow = np.concatenate([ow, np.zeros((D, VSP - VSR), np.float32)], axis=1)
        m["outw"] = _f16(ow.reshape(NDC, 128, NVC, 128).transpose(2, 1, 0, 3))
        ob = np.full(VSP, -30.0, np.float32)
        ob[:VSR] = inp["out_b"][c * VSR:(c + 1) * VSR]
        m["outb"] = _f32(ob.reshape(NVC, 128).T)
        in_maps.append(m)
    return in_maps


_NC_CACHE = {}


def kernel(**inputs):
    import os
    inputs = {k: np.asarray(v, dtype=np.float32) for k, v in inputs.items()}
    if "nc" not in _NC_CACHE:
        _NC_CACHE["nc"] = build_bass()
    nc = _NC_CACHE["nc"]
    in_maps = prepare_inputs(inputs)
    trace = bool(int(os.environ.get("KB_TRACE", "0")))
    tmpdir = os.environ.get("KB_TMPDIR") or None
    res = run_bass_kernel_spmd(nc, in_maps, list(range(NCORES)), trace=trace,
                               tmpdir=tmpdir)
    LAST_RESULTS["res"] = res
    shards = [np.asarray(res.results[c]["probsT"][:VSR], dtype=np.float32)
              for c in range(NCORES)]
    return np.ascontiguousarray(np.concatenate(shards, axis=0).T)


# revision 34
# speedup vs baseline: 1.0301x; 1.0119x over previous
"""Trainium2 Bass kernel for nn_LonelyDecoder (dense transformer, 8-core TP).

v6 highlights:
 - fp16 everywhere (same PE rate as bf16, ~8x less rounding noise).
 - embW resident in SBUF; per-s-chunk activation tiles (no whole-tile
   false deps across pipeline stages).
 - One software-pipelined schedule with lag-2 stages: MHA2 chunk sc |
   ln2 chunk sc-1 | FFN chunk sc-2, and ln3(sc+1) injected mid
   output-GEMM(sc), so the LN scalar chain (DVE smalls + gpsimd
   broadcast) never sits on the PE critical path.
 - LN rsqrt computed on DVE via bitcast+Newton (no ACT Sqrt table-set
   switches); all broadcasts via gpsimd PartitionBroadcast; gpsimd kept
   extended-lib-only (library reloads cost ~8us).
 - Output GEMM per s-chunk with pipelined softmax-sum AllReduce; only
   the last chunk's AR + normalize + writeout is exposed.
"""

import numpy as np
import ml_dtypes

import concourse.bacc as bacc
import concourse.bass as bass
import concourse.mybir as mybir
import concourse.tile as tile
from concourse.bass_utils import run_bass_kernel_spmd

F32 = mybir.dt.float32
F16 = mybir.dt.float16
I32 = mybir.dt.int32
AF = mybir.ActivationFunctionType
ALU = mybir.AluOpType

S, V, D, H, DK, DFF, L = 2048, 32000, 1024, 16, 64, 4096, 4
NCORES = 8
VSR = V // NCORES          # 4000 real vocab shard
VSP = 4096                 # padded vocab shard (32 x 128)
NVC = VSP // 128           # 32 v-chunks
NDC = D // 128             # 8 d-chunks
NSC = 4                    # s-chunks of 512
SC = 512
NTT = S // 128             # 16 t-tiles
FS = DFF // NCORES         # 512 ff shard
NFC = FS // 128            # 4 ff chunks
RG = [list(range(NCORES))]

LAST_RESULTS = {}


def ts(i, n):
    return slice(i * n, (i + 1) * n)


def build_bass():
    nc = bacc.Bacc(None, target_bir_lowering=False)

    # ---- I/O ----
    xT = nc.dram_tensor("xT", [VSP, S], F16, kind="ExternalInput")
    embW = nc.dram_tensor("embW", [VSP, D], F16, kind="ExternalInput")
    pebT = nc.dram_tensor("pebT", [D, S], F16, kind="ExternalInput")
    qkw = [nc.dram_tensor(f"qkw{m}", [128, NDC, 256], F16, kind="ExternalInput") for m in (1, 2)]
    bqk = [nc.dram_tensor(f"bqk{m}", [128, 2], F32, kind="ExternalInput") for m in (1, 2)]
    vw = [nc.dram_tensor(f"vw{m}", [128, NDC, 130], F16, kind="ExternalInput") for m in (1, 2)]
    bv = [nc.dram_tensor(f"bv{m}", [128, 1], F32, kind="ExternalInput") for m in (1, 2)]
    maskT = nc.dram_tensor("maskT", [128, 4 * SC], F16, kind="ExternalInput")
    f1w = nc.dram_tensor("f1w", [128, NDC, FS], F16, kind="ExternalInput")
    f1b = nc.dram_tensor("f1b", [128, NFC], F32, kind="ExternalInput")
    f2w = nc.dram_tensor("f2w", [128, NFC, D], F16, kind="ExternalInput")
    f2bT = nc.dram_tensor("f2bT", [128, NDC], F32, kind="ExternalInput")
    lngT = nc.dram_tensor("lngT", [128, NDC], F32, kind="ExternalInput")
    lnbT = nc.dram_tensor("lnbT", [128, NDC], F32, kind="ExternalInput")
    outw = nc.dram_tensor("outw", [NVC, 128, NDC, 128], F16, kind="ExternalInput")
    outb = nc.dram_tensor("outb", [128, NVC], F32, kind="ExternalInput")
    probsT = nc.dram_tensor("probsT", [VSP, S], F16, kind="ExternalOutput")

    with tile.TileContext(nc) as tc:
        with tc.tile_pool(name="dram", bufs=1, space="DRAM") as dram, \
             tc.tile_pool(name="const", bufs=1) as const:

            # internal DRAM (collective bounce buffers)
            h_par = [dram.tile([D, SC], F16, tag=f"hp{sc}", name=f"h_par{sc}")
                     for sc in range(NSC)]
            h_red = [dram.tile([D, SC], F16, tag=f"hr{sc}", addr_space="Shared",
                               name=f"h_red{sc}") for sc in range(NSC)]
            a_in = [[dram.tile([128, SC], F16, tag=f"a{m}i{sc}", name=f"a{m}_in{sc}")
                     for sc in range(NSC)] for m in (0, 1)]
            a_out = [[dram.tile([D, SC], F16, tag=f"a{m}o{sc}", addr_space="Shared",
                                name=f"a{m}_out{sc}") for sc in range(NSC)]
                     for m in (0, 1)]
            y_par = [dram.tile([D, SC], F16, tag=f"yp{sc}", name=f"y_par{sc}")
                     for sc in range(NSC)]
            y_red = [dram.tile([D, SC], F16, tag=f"yr{sc}", addr_space="Shared",
                               name=f"y_red{sc}") for sc in range(NSC)]
            ss_in = [dram.tile([1, SC], F32, tag=f"ssi{sc}", name=f"ss_in{sc}")
                     for sc in range(NSC)]
            ss_out = [dram.tile([1, SC], F32, tag=f"sso{sc}", addr_space="Shared",
                                name=f"ss_out{sc}") for sc in range(NSC)]

            # constants (scalar DMA queue; sync stays free for GEMM tiles)
            ones_col = const.tile([128, 1], F16, tag="c1")
            nc.vector.memset(ones_col[:, :], 1.0)
            ones_row = const.tile([1, 128], F16, tag="c3")
            nc.vector.memset(ones_row[:, :], 1.0)
            bqk_sb = [const.tile([128, 2], F32, tag=f"bqk{m}", name=f"bqk_sb{m}") for m in range(2)]
            bv_sb = [const.tile([128, 1], F32, tag=f"bv{m}", name=f"bv_sb{m}") for m in range(2)]
            for m in range(2):
                nc.scalar.dma_start(bqk_sb[m][:, :], bqk[m][:, :])
                nc.scalar.dma_start(bv_sb[m][:, :], bv[m][:, :])
            f1b_sb = const.tile([128, NFC], F32, tag="f1b")
            nc.scalar.dma_start(f1b_sb[:, :], f1b[:, :])
            f2bT_sb = const.tile([128, NDC], F32, tag="f2bT")
            nc.scalar.dma_start(f2bT_sb[:, :], f2bT[:, :])
            lng_sb = const.tile([128, NDC], F32, tag="lng")
            nc.scalar.dma_start(lng_sb[:, :], lngT[:, :])
            lnb_sb = const.tile([128, NDC], F32, tag="lnb")
            nc.scalar.dma_start(lnb_sb[:, :], lnbT[:, :])
            outb_sb = const.tile([128, NVC], F32, tag="outb")
            nc.scalar.dma_start(outb_sb[:, :], outb[:, :])

            # per-s-chunk activation tiles: tag per sc so cross-chunk
            # consumers never pick up whole-tile false dependencies.
            acts_ctx = tc.tile_pool(name="acts", bufs=2)
            acts = acts_ctx.__enter__()

            def act_tiles(name):
                return [acts.tile([128, NDC, SC], F16, tag=f"act{sc}",
                                  name=f"{name}{sc}") for sc in range(NSC)]

            hT = act_tiles("hT")

            # ---------- phase E: embedding GEMM (chunked AllReduce) ----------
            with tc.tile_pool(name="embw", bufs=1) as embp, \
                 tc.tile_pool(name="xt", bufs=6) as xtp, \
                 tc.tile_pool(name="peb", bufs=1) as pebp, \
                 tc.tile_pool(name="ps_e", bufs=1, space="PSUM") as pse, \
                 tc.tile_pool(name="ev_e", bufs=3) as evp, \
                 tc.tile_pool(name="addin_e", bufs=3) as adpe:
                # embW resident (64KB/partition), loaded once on scalar queue
                ew_sb = embp.tile([128, NVC, D], F16, tag="ew")
                peb_sb = pebp.tile([128, NDC, S], F16, tag="peb")
                for dc in range(NDC):
                    nc.gpsimd.dma_start(peb_sb[:, dc, :], pebT[ts(dc, 128), :])

                def ht_prep(psc):
                    for dc in range(NDC):
                        hr = adpe.tile([128, SC], F16, tag="addin",
                                       name=f"hr_{psc}_{dc}")
                        nc.scalar.dma_start(hr[:, :], h_red[psc][ts(dc, 128), :])
                        nc.vector.tensor_add(hT[psc][:, dc, :], hr[:, :],
                                             peb_sb[:, dc, ts(psc, SC)])

                for sc in range(NSC):
                    with nc.named_scope(f"E{sc}"):
                        pes = [pse.tile([128, SC], F32, tag=f"pe{dc}",
                                        name=f"pe_{sc}_{dc}")
                               for dc in range(NDC)]
                        for kc in range(NVC):
                            xt = xtp.tile([128, SC], F16, tag="xt")
                            nc.sync.dma_start(xt[:, :], xT[ts(kc, 128), ts(sc, SC)])
                            if sc == 0:
                                nc.scalar.dma_start(ew_sb[:, kc, :],
                                                    embW[ts(kc, 128), :])
                            for dc in range(NDC):
                                nc.tensor.matmul(
                                    pes[dc][:, :],
                                    ew_sb[:, kc, ts(dc, 128)],
                                    xt[:, :],
                                    start=(kc == 0),
                                    stop=(kc == NVC - 1),
                                )
                        for dc in range(NDC):
                            hv = evp.tile([128, SC], F16, tag="ev")
                            nc.scalar.activation(hv[:, :], pes[dc][:, :], AF.Copy)
                            nc.sync.dma_start(h_par[sc][ts(dc, 128), :], hv[:, :])
                        nc.gpsimd.collective_compute(
                            "AllReduce", ALU.add, replica_groups=RG,
                            ins=[h_par[sc][:, :].opt()],
                            outs=[h_red[sc][:, :].opt()],
                        )
                        if sc > 0:
                            ht_prep(sc - 1)
                ht_prep(NSC - 1)

            # ======== pipelined layer ========
            # unified PSUM pool (8 banks): ps1(2) + po(2) + pg(2x2=4)
            with tc.tile_pool(name="addin", bufs=3) as adp, \
                 tc.tile_pool(name="x2p", bufs=2) as x2p, \
                 tc.tile_pool(name="ev_a", bufs=2) as evp, \
                 tc.tile_pool(name="small_a", bufs=2) as smp, \
                 tc.tile_pool(name="osc", bufs=2) as osc, \
                 tc.tile_pool(name="pp", bufs=3) as ppp, \
                 tc.tile_pool(name="outwp", bufs=2) as owp, \
                 tc.tile_pool(name="ffw", bufs=1) as ffp, \
                 tc.tile_pool(name="exp", bufs=2) as expp, \
                 tc.tile_pool(name="ps_a", bufs=2, space="PSUM") as psa:

                def mha_proj_chunk(mi, sc, actT, qkw_sb, vw_sb, V_sb, qT2, kT2):
                    """QKV projections for weight-set mi, one s-chunk."""
                    for tt in range(4 * sc, 4 * sc + 4):
                        pv = psa.tile([128, SC], F32, tag="ps1", name=f"pv{mi}_{tt}")
                        for dc in range(NDC):
                            nc.tensor.matmul(
                                pv[:, 0:130], actT[sc][:, dc, ts(tt % 4, 128)],
                                vw_sb[:, dc, :],
                                start=(dc == 0), stop=(dc == NDC - 1),
                            )
                        nc.scalar.activation(V_sb[:, tt, :], pv[:, 0:130], AF.Copy)
                    for wi, dst in ((0, qT2), (1, kT2)):
                        pq = psa.tile([128, SC], F32, tag="ps1", name=f"pq{mi}_{wi}_{sc}")
                        for dc in range(NDC):
                            nc.tensor.matmul(
                                pq[:, :],
                                qkw_sb[:, dc, ts(wi, 128)],
                                actT[sc][:, dc, :],
                                start=(dc == 0), stop=(dc == NDC - 1),
                            )
                        nc.scalar.activation(
                            dst[:, sc, :], pq[:, :], AF.Identity,
                            bias=bqk_sb[mi][:, wi:wi + 1],
                        )
                    nc.vector.memset(V_sb[:, ts(sc, 4), 64:65], 1.0)
                    nc.vector.memset(V_sb[:, ts(sc, 4), 129:130], 1.0)

                def mha_chunk(mi, sc, masked, attnT, V_sb, qT2, kT2, mask_sb):
                    """scores+AV+normalize for one s-chunk, both heads, then
                    a_in DMA + AllGather."""
                    for h in range(2):
                        po = psa.tile([128, SC], F32, tag="po", name=f"po{mi}_{h}_{sc}")
                        tts = list(range(4 * (sc + 1))) if masked else list(range(NTT))
                        pairs = [tts[i:i + 2] for i in range(0, len(tts), 2)]
                        for pi, pr in enumerate(pairs):
                            pg = psa.tile([128, 2 * SC], F32, tag="pg",
                                          name=f"pg{mi}_{h}_{sc}_{pi}")
                            for j, tt in enumerate(pr):
                                nc.tensor.matmul(
                                    pg[:, ts(j, SC)],
                                    kT2[ts(h, 64), tt // 4, ts(tt % 4, 128)],
                                    qT2[ts(h, 64), sc, :],
                                    start=True, stop=True,
                                )
                            et = evp.tile([128, 2 * SC], F16, tag="exp")
                            nc.scalar.activation(et[:, :], pg[:, :], AF.Exp,
                                                 scale=1.0 / D)
                            if masked and pr[0] >= 4 * sc:
                                mo = (pr[0] - 4 * sc) * SC
                                nc.vector.tensor_mul(
                                    et[:, :], et[:, :],
                                    mask_sb[:, mo:mo + 2 * SC],
                                )
                            for j, tt in enumerate(pr):
                                nc.tensor.matmul(
                                    po[0:65, :],
                                    V_sb[:, tt, ts(h, 65)],
                                    et[:, ts(j, SC)],
                                    start=(pi == 0 and j == 0),
                                    stop=(pi == len(pairs) - 1 and j == len(pr) - 1),
                                )
                        oo = smp.tile([64, SC], F16, tag="oo", bufs=1, name=f"oo{mi}_{h}_{sc}")
                        nc.scalar.activation(oo[:, :], po[0:64, :], AF.Copy)
                        s0 = smp.tile([1, SC], F32, tag="s0", bufs=1, name=f"s0{mi}_{h}_{sc}")
                        nc.scalar.activation(s0[:, :], po[64:65, :], AF.Copy)
                        rec = smp.tile([1, SC], F32, tag="rec", bufs=1, name=f"rec{mi}_{h}_{sc}")
                        nc.vector.reciprocal_approx_fast(rec[:, :], s0[:, :])
                        rec16 = smp.tile([1, SC], F16, tag="rec16", bufs=1, name=f"rec16_{mi}_{h}_{sc}")
                        nc.vector.tensor_copy(rec16[:, :], rec[:, :])
                        rb = smp.tile([64, SC], F16, tag="rb", bufs=1, name=f"rb{mi}_{h}_{sc}")
                        nc.gpsimd.partition_broadcast(rb[:, :], rec16[0:1, :])
                        nc.vector.tensor_mul(oo[:, :], oo[:, :], rb[:, :])
                        nc.scalar.activation(
                            attnT[ts(h, 64), sc, :], oo[:, :], AF.Identity,
                            bias=bv_sb[mi][ts(h, 64), :],
                        )
                    nc.sync.dma_start(a_in[mi][sc][:, :], attnT[:, sc, :])
                    nc.gpsimd.collective_compute(
                        "AllGather", ALU.bypass, replica_groups=RG,
                        ins=[a_in[mi][sc][:, :].opt()],
                        outs=[a_out[mi][sc][:, :].opt()],
                    )

                # residual + layernorm over the feature dim for ONE s-chunk.
                # prevT/newT are lists of per-sc tiles [128, NDC, SC].
                def ln_chunk(prevT, newT, sc, addin_fn, name):
                    stats = psa.tile([65, SC], F32, tag="ps1",
                                     name=f"st_{name}_{sc}")
                    for dc in range(NDC):
                        src_ap, xbias = addin_fn(sc, dc)
                        ad = adp.tile([128, SC], F16, tag="addin",
                                      name=f"ad_{name}_{sc}_{dc}")
                        nc.scalar.dma_start(ad[:, :], src_ap)
                        if xbias is not None:
                            nc.vector.scalar_tensor_tensor(
                                prevT[sc][:, dc, :], ad[:, :], xbias,
                                prevT[sc][:, dc, :], op0=ALU.add, op1=ALU.add)
                        else:
                            nc.vector.tensor_add(prevT[sc][:, dc, :],
                                                 prevT[sc][:, dc, :], ad[:, :])
                        x2 = x2p.tile([128, SC], F16, tag="x2",
                                      name=f"x2_{name}_{sc}_{dc}")
                        nc.vector.tensor_mul(x2[:, :], prevT[sc][:, dc, :],
                                             prevT[sc][:, dc, :])
                        nc.tensor.matmul(stats[0:1, :], ones_col[:, :],
                                         prevT[sc][:, dc, :],
                                         start=(dc == 0), stop=(dc == NDC - 1))
                        nc.tensor.matmul(stats[64:65, :], ones_col[:, :],
                                         x2[:, :],
                                         start=(dc == 0), stop=(dc == NDC - 1))
                    nm = smp.tile([1, SC], F32, tag="nm", bufs=1, name=f"nm_{name}_{sc}")
                    nc.vector.tensor_scalar_mul(nm[:, :], stats[0:1, :], -1.0 / D)
                    # e2 = E[x^2] + eps - mu^2  (variance + eps)
                    e2 = smp.tile([1, SC], F32, tag="e2", bufs=1, name=f"e2_{name}_{sc}")
                    nc.vector.tensor_scalar(e2[:, :], stats[64:65, :], 1.0 / D,
                                            1e-5, op0=ALU.mult, op1=ALU.add)
                    musq = smp.tile([1, SC], F32, tag="musq", bufs=1, name=f"musq_{name}_{sc}")
                    nc.vector.tensor_mul(musq[:, :], nm[:, :], nm[:, :])
                    nc.vector.tensor_sub(e2[:, :], e2[:, :], musq[:, :])
                    # inv = rsqrt(e2) on DVE: quake seed + 2 Newton steps
                    # (no ACT Sqrt -> no table-set switch)
                    yi = smp.tile([1, SC], I32, tag="yi", bufs=1,
                                  name=f"yi_{name}_{sc}")
                    nc.vector.tensor_scalar(yi[:, :], e2[:, :].bitcast(I32),
                                            1, None, op0=ALU.logical_shift_right)
                    nc.vector.tensor_scalar(yi[:, :], yi[:, :], -1,
                                            None, op0=ALU.bitwise_xor)
                    nc.vector.tensor_scalar(yi[:, :], yi[:, :], 0x5f3759e0,
                                            None, op0=ALU.add)
                    inv = yi[:, :].bitcast(F32)
                    for it in range(2):
                        h2c = smp.tile([1, SC], F32, tag="h2c", bufs=1,
                                       name=f"h2c_{name}_{sc}_{it}")
                        nc.vector.tensor_mul(h2c[:, :], inv, inv)
                        nc.vector.tensor_mul(h2c[:, :], h2c[:, :], e2[:, :])
                        nc.vector.tensor_scalar(h2c[:, :], h2c[:, :], -0.5, 1.5,
                                                op0=ALU.mult, op1=ALU.add)
                        nc.vector.tensor_mul(inv, inv, h2c[:, :])
                    ninv = smp.tile([1, SC], F32, tag="ninv", bufs=1, name=f"ninv_{name}_{sc}")
                    nc.vector.tensor_mul(ninv[:, :], nm[:, :], inv)
                    inv16 = smp.tile([1, 2 * SC], F16, tag="inv16", bufs=1, name=f"inv16_{name}_{sc}")
                    nc.vector.tensor_copy(inv16[:, 0:SC], inv)
                    nc.vector.tensor_copy(inv16[:, SC:2 * SC], ninv[:, :])
                    bb = x2p.tile([128, 2 * SC], F16, tag="bb", bufs=1,
                                  name=f"bb_{name}_{sc}")
                    nc.gpsimd.partition_broadcast(bb[:, :], inv16[0:1, :])
                    for dc in range(NDC):
                        t1 = x2p.tile([128, SC], F16, tag="t1",
                                      name=f"t1_{name}_{sc}_{dc}", bufs=2)
                        nc.vector.tensor_mul(t1[:, :], prevT[sc][:, dc, :],
                                             bb[:, 0:SC])
                        nc.vector.tensor_add(t1[:, :], t1[:, :], bb[:, SC:2 * SC])
                        nc.vector.tensor_scalar(newT[sc][:, dc, :], t1[:, :],
                                                lng_sb[:, dc:dc + 1],
                                                lnb_sb[:, dc:dc + 1],
                                                op0=ALU.mult, op1=ALU.add)

                def attn_addin(mi):
                    def fn(sc, dc):
                        return (a_out[mi][sc][ts(dc, 128), :], None)
                    return fn

                def y_addin(sc, dc):
                    return (y_red[sc][ts(dc, 128), :], f2bT_sb[:, dc:dc + 1])

                # FFN weights resident; loaded early on scalar queue
                f1w_sb = ffp.tile([128, NDC, FS], F16, tag="f1w")
                nc.scalar.dma_start(f1w_sb[:, :, :], f1w[:, :, :])
                f2w_sb = ffp.tile([128, NFC, D], F16, tag="f2w")
                nc.scalar.dma_start(f2w_sb[:, :, :], f2w[:, :, :])

                def ffn_ar(sc):
                    nc.gpsimd.collective_compute(
                        "AllReduce", ALU.add, replica_groups=RG,
                        ins=[y_par[sc][:, :].opt()], outs=[y_red[sc][:, :].opt()],
                    )

                def ffn_chunk(h2T, sc):
                    uT = ffp.tile([128, NFC, SC], F16, tag="uT", bufs=1,
                                  name=f"uT_{sc}")
                    for fc in range(NFC):
                        pu = psa.tile([128, SC], F32, tag="ps1", name=f"pu_{fc}_{sc}")
                        for dc in range(NDC):
                            nc.tensor.matmul(pu[:, :], f1w_sb[:, dc, ts(fc, 128)],
                                             h2T[sc][:, dc, :],
                                             start=(dc == 0), stop=(dc == NDC - 1))
                        nc.scalar.activation(uT[:, fc, :], pu[:, :], AF.Relu,
                                             bias=f1b_sb[:, fc:fc + 1])
                    for dc in range(NDC):
                        py = psa.tile([128, SC], F32, tag="ps1", name=f"py_{dc}_{sc}")
                        for fc in range(NFC):
                            nc.tensor.matmul(py[:, :], f2w_sb[:, fc, ts(dc, 128)],
                                             uT[:, fc, :],
                                             start=(fc == 0), stop=(fc == NFC - 1))
                        yv = evp.tile([128, SC], F16, tag="yv", bufs=2,
                                      name=f"yv_{dc}_{sc}")
                        nc.scalar.activation(yv[:, :], py[:, :], AF.Copy)
                        nc.sync.dma_start(y_par[sc][ts(dc, 128), :], yv[:, :])

                # ---- output GEMM + softmax machinery (per s-chunk) ----
                # exp-tile buffering: tail(sc) frees eo_[sc][vc] at
                # ~AR-latency into gemm(sc+1); earlier v-chunks need a
                # second generation.
                ETC = 13
                ets = [[expp.tile([128, SC], F16, tag=f"eo_{vc}",
                                  bufs=(2 if vc < ETC else 1),
                                  name=f"eo_{sc}_{vc}")
                        for vc in range(NVC)] for sc in range(NSC)]

                def wvt_load(sc, vc):
                    wvt = owp.tile([128, NDC, 128], F16, tag="ow",
                                   name=f"ow_{sc}_{vc}")
                    nc.scalar.dma_start(wvt[:, :, :], outw[vc, :, :, :])
                    return wvt

                def out_gemm_chunk(outT, sc, wvt0, injects=()):
                    injects = dict(injects)
                    pss = psa.tile([65, SC], F32, tag="ps1", name=f"pss_{sc}")
                    wvts = {0: wvt0}
                    for vc in range(NVC):
                        if vc + 1 < NVC:
                            wvts[vc + 1] = wvt_load(sc, vc + 1)
                        pl = psa.tile([128, SC], F32, tag="po",
                                      name=f"pl_{sc}_{vc}")
                        for dc in range(NDC):
                            nc.tensor.matmul(pl[:, :], wvts[vc][:, dc, :],
                                             outT[sc][:, dc, :],
                                             start=(dc == 0), stop=(dc == NDC - 1))
                        nc.scalar.activation(ets[sc][vc][:, :], pl[:, :],
                                             AF.Exp, bias=outb_sb[:, vc:vc + 1])
                        nc.tensor.matmul(pss[0:1, :], ones_col[:, :],
                                         ets[sc][vc][:, :],
                                         start=(vc == 0), stop=(vc == NVC - 1))
                        del wvts[vc]
                        if vc in injects:
                            injects[vc]()
                    sv = smp.tile([1, SC], F32, tag="ssv", bufs=1, name=f"ssv_{sc}")
                    nc.scalar.activation(sv[:, :], pss[0:1, :], AF.Copy)
                    nc.scalar.dma_start(ss_in[sc][0:1, :], sv[:, :])
                    nc.gpsimd.collective_compute(
                        "AllReduce", ALU.add, replica_groups=RG,
                        ins=[ss_in[sc][:, :].opt()], outs=[ss_out[sc][:, :].opt()],
                    )

                def out_tail_pre(sc):
                    # reciprocal of the AllReduced exp-sums; off the PE/scalar
                    # critical queues so nothing stalls waiting for the AR.
                    rr = osc.tile([1, SC], F32, tag="rr", bufs=1, name=f"rr{sc}")
                    nc.sync.dma_start(rr[:, :], ss_out[sc][0:1, :])
                    ri = osc.tile([1, SC], F32, tag="ri", bufs=1, name=f"ri{sc}")
                    nc.vector.reciprocal_approx_fast(ri[:, :], rr[:, :])
                    ri16 = osc.tile([1, SC], F16, tag="ri16", bufs=1, name=f"ri16_{sc}")
                    nc.vector.tensor_copy(ri16[:, :], ri[:, :])
                    return ri16

                def out_tail_post(sc, ri16, last=False):
                    # emitted mid-gemm(sc+1), after the AR has landed; all
                    # muls on DVE (gpsimd stays extended-lib-only), probs
                    # writeout on sync; late v-chunks first (their
                    # single-buffered exp tiles gate gemm(sc+1)). For the
                    # final chunk the broadcast runs on the (now idle) PE
                    # and the muls read PSUM directly -- shortest chain.
                    hsl = ts(sc, SC)
                    if last:
                        recb = psa.tile([128, SC], F32, tag="po",
                                        name=f"recbp{sc}")
                        nc.tensor.matmul(recb[:, :], ones_row[:, :],
                                         ri16[0:1, :], start=True, stop=True)
                        rb_ap = recb[:, :]
                    else:
                        recb_sb = osc.tile([128, SC], F16, tag="recb", bufs=1,
                                           name=f"recb_sb{sc}")
                        nc.gpsimd.partition_broadcast(recb_sb[:, :], ri16[0:1, :])
                        rb_ap = recb_sb[:, :]
                    for vc in list(range(ETC, NVC)) + list(range(ETC)):
                        pp = ppp.tile([128, SC], F16, tag="ppv",
                                      name=f"pp_{vc}_{sc}")
                        nc.vector.tensor_mul(pp[:, :], ets[sc][vc][:, :], rb_ap)
                        nc.sync.dma_start(probsT[ts(vc, 128), hsl], pp[:, :])

                # ================= emission schedule =================
                with tc.tile_pool(name="attn", bufs=1) as attnp:
                    qkw_sbs, vw_sbs = [], []
                    for mi in range(2):
                        qs = attnp.tile([128, NDC, 256], F16, tag="qkw",
                                        name=f"qkw_sb{mi}")
                        nc.scalar.dma_start(qs[:, :, :], qkw[mi][:, :, :])
                        vs = attnp.tile([128, NDC, 130], F16, tag="vw",
                                        name=f"vw_sb{mi}")
                        nc.scalar.dma_start(vs[:, :, :], vw[mi][:, :, :])
                        qkw_sbs.append(qs)
                        vw_sbs.append(vs)

                    with tc.tile_pool(name="maskp", bufs=1) as maskp:
                        mask_sb = maskp.tile([128, 4 * SC], F16, tag="mask")
                        nc.scalar.dma_start(mask_sb[:, :], maskT[:, :])

                        V1 = attnp.tile([128, NTT, 130], F16, tag="V", name="V_sb0")
                        q1 = attnp.tile([128, NSC, SC], F16, tag="qT2", name="qT2_0")
                        k1 = attnp.tile([128, NSC, SC], F16, tag="kT2", name="kT2_0")
                        at1 = attnp.tile([128, NSC, SC], F16, tag="attnT",
                                         name="attnT0")

                        # --- MHA1 (masked) + ln1, pipelined per s-chunk.
                        # Projections hoisted ahead of the (short, latency-
                        # bound) masked chunks to keep the PE fed. ---
                        h1T = act_tiles("h1T")
                        with nc.named_scope("A1c0"):
                            mha_proj_chunk(0, 0, hT, qkw_sbs[0], vw_sbs[0],
                                           V1, q1, k1)
                            mha_proj_chunk(0, 1, hT, qkw_sbs[0], vw_sbs[0],
                                           V1, q1, k1)
                            mha_chunk(0, 0, True, at1, V1, q1, k1, mask_sb)
                        with nc.named_scope("A1c1"):
                            mha_proj_chunk(0, 2, hT, qkw_sbs[0], vw_sbs[0],
                                           V1, q1, k1)
                            mha_chunk(0, 1, True, at1, V1, q1, k1, mask_sb)
                        with nc.named_scope("A1l0"):
                            ln_chunk(hT, h1T, 0, attn_addin(0), "h1T")
                        with nc.named_scope("A1c2"):
                            mha_proj_chunk(0, 3, hT, qkw_sbs[0], vw_sbs[0],
                                           V1, q1, k1)
                            mha_chunk(0, 2, True, at1, V1, q1, k1, mask_sb)
                        with nc.named_scope("A1l1"):
                            ln_chunk(hT, h1T, 1, attn_addin(0), "h1T")
                        with nc.named_scope("A1c3"):
                            mha_chunk(0, 3, True, at1, V1, q1, k1, mask_sb)
                        with nc.named_scope("A1l2"):
                            ln_chunk(hT, h1T, 2, attn_addin(0), "h1T")

                    # --- MHA2 (unmasked) + ln2 + FFN, lag-2 pipeline.
                    # FFN AllReduces are deferred until after the last
                    # AllGather so the latency-critical gathers never queue
                    # behind them on the collective engine. ---
                    V2 = attnp.tile([128, NTT, 130], F16, tag="V", name="V_sb1")
                    q2 = attnp.tile([128, NSC, SC], F16, tag="qT2", name="qT2_1")
                    k2 = attnp.tile([128, NSC, SC], F16, tag="kT2", name="kT2_1")
                    at2 = attnp.tile([128, NSC, SC], F16, tag="attnT", name="attnT1")
                    for sc in range(3):
                        with nc.named_scope(f"A2p{sc}"):
                            mha_proj_chunk(1, sc, h1T, qkw_sbs[1], vw_sbs[1],
                                           V2, q2, k2)
                    with nc.named_scope("A1l3"):
                        ln_chunk(hT, h1T, NSC - 1, attn_addin(0), "h1T")
                    with nc.named_scope("A2p3"):
                        mha_proj_chunk(1, 3, h1T, qkw_sbs[1], vw_sbs[1],
                                       V2, q2, k2)
                    h2T = act_tiles("h2T")
                    outT = act_tiles("outT")
                    # chunks + ln2 only: the FFN work (and its AllReduces,
                    # whose inputs becoming ready would steal the collective
                    # engine from the latency-critical gathers) runs after
                    # the last AllGather is in flight.
                    for sc in range(NSC):
                        with nc.named_scope(f"A2c{sc}"):
                            mha_chunk(1, sc, False, at2, V2, q2, k2, None)
                        if sc > 0:
                            with nc.named_scope(f"A2l{sc-1}"):
                                ln_chunk(h1T, h2T, sc - 1, attn_addin(1), "h2T")
                        if sc == 3:
                            with nc.named_scope("A2f0"):
                                ffn_chunk(h2T, 0)
                                ffn_ar(0)
                    with nc.named_scope("A2l3"):
                        ln_chunk(h1T, h2T, NSC - 1, attn_addin(1), "h2T")
                    with nc.named_scope("A2f1"):
                        ffn_chunk(h2T, 1)
                        ffn_ar(1)
                    with nc.named_scope("A2f2"):
                        ffn_chunk(h2T, 2)
                        ffn_ar(2)
                    with nc.named_scope("Ol0"):
                        ln_chunk(h2T, outT, 0, y_addin, "outT")
                    with nc.named_scope("A2f3"):
                        ffn_chunk(h2T, 3)
                        ffn_ar(3)
                    with nc.named_scope("Ol1"):
                        ln_chunk(h2T, outT, 1, y_addin, "outT")

                    # --- output GEMM + softmax, ln3(sc+1) and tail(sc-1)
                    #     injected mid-gemm(sc) so the PE never waits ---
                    for sc in range(NSC):
                        wvt0 = wvt_load(sc, 0)
                        injects = []
                        if sc + 2 < NSC:
                            # ln3 of the chunk-after-next, early in the vc
                            # loop (its FFN AR has long landed)
                            injects.append(
                                (2, lambda n=sc + 2: ln_chunk(
                                    h2T, outT, n, y_addin, "outT")))
                        if sc > 0:
                            # tail_pre emitted WITH tail_post at vc=ETC so
                            # its AR-gated reciprocal never head-blocks the
                            # DVE queue ahead of PE-critical LN work
                            def tail(p=sc - 1):
                                out_tail_post(p, out_tail_pre(p))
                            injects.append((ETC, tail))
                        with nc.named_scope(f"Og{sc}"):
                            out_gemm_chunk(outT, sc, wvt0, injects=injects)
                    with nc.named_scope("Ot3"):
                        out_tail_post(NSC - 1, out_tail_pre(NSC - 1), last=True)

            acts_ctx.__exit__(None, None, None)

    nc.compile()
    return nc


def _positional_encoding():
    pos = np.arange(S, dtype=np.float32)[:, None]
    i = np.arange(0, D, 2, dtype=np.float32)
    ang = (pos * np.exp((-np.log(10000.0) * i / D).astype(np.float32))).astype(np.float32)
    pe = np.zeros((S, D), np.float32)
    pe[:, 0::2] = np.sin(ang)
    pe[:, 1::2] = np.cos(ang)
    return pe


def _f16(x):
    return np.ascontiguousarray(x).astype(np.float16)


def _f32(x):
    return np.ascontiguousarray(x, dtype=np.float32)


def prepare_inputs(inp):
    """Full fp32 inputs -> per-core input maps (host-side sharding/layout)."""
    li = L - 1
    xT_full = np.ascontiguousarray(inp["x"].T)          # [V, S]
    peb = (inp["emb_b"][None, :] + _positional_encoding()).astype(np.float32)
    pebT = _f16(peb.T)                                   # [D, S] fp16

    # causal mask patterns for the 4 diagonal t-tiles of an s-chunk
    t_loc = np.arange(128)[:, None]
    s_loc = np.arange(SC)[None, :]
    maskT = np.concatenate(
        [((p * 128 + t_loc) <= s_loc).astype(np.float32) for p in range(4)], axis=1
    )
    maskT = _f16(maskT)                                  # [128, 2048]

    lngT = _f32(inp["ln_g"].reshape(NDC, 128).T)
    lnbT = _f32(inp["ln_b"].reshape(NDC, 128).T)
    f2bT = _f32(inp["ff_b2"][li].reshape(NDC, 128).T)

    in_maps = []
    for c in range(NCORES):
        m = {}
        xs = xT_full[c * VSR:(c + 1) * VSR]              # [4000, S]
        m["xT"] = _f16(np.concatenate([xs, np.zeros((VSP - VSR, S), np.float32)], 0))
        ew = inp["emb_W"][c * VSR:(c + 1) * VSR]
        m["embW"] = _f16(np.concatenate([ew, np.zeros((VSP - VSR, D), np.float32)], 0))
        m["pebT"] = pebT
        m["maskT"] = maskT
        for mi, (Wq, bq, Wk, bk, Wv, bvv) in enumerate([
            (inp["Wq1"][li], inp["bq1"][li], inp["Wk1"][li], inp["bk1"][li],
             inp["Wv1"][li], inp["bv1"][li]),
            (inp["Wq2"][li], inp["bq2"][li], inp["Wk2"][li], inp["bk2"][li],
             inp["Wv2"][li], inp["bv2"][li]),
        ]):
            h0, h1 = 2 * c, 2 * c + 1
            qk = np.concatenate([Wq[h0], Wq[h1], Wk[h0], Wk[h1]], axis=1)  # [D, 256]
            m[f"qkw{mi+1}"] = _f16(qk.reshape(NDC, 128, 256).transpose(1, 0, 2))
            m[f"bqk{mi+1}"] = _f32(np.stack(
                [np.concatenate([bq[h0], bq[h1]]),
                 np.concatenate([bk[h0], bk[h1]])], axis=1))
            vp = np.zeros((D, 130), np.float32)
            vp[:, 0:64] = Wv[h0]
            vp[:, 65:129] = Wv[h1]
            m[f"vw{mi+1}"] = _f16(vp.reshape(NDC, 128, 130).transpose(1, 0, 2))
            m[f"bv{mi+1}"] = _f32(np.concatenate([bvv[h0], bvv[h1]])[:, None])
        w1 = inp["ff_W1"][li][:, c * FS:(c + 1) * FS]    # [D, FS]
        m["f1w"] = _f16(w1.reshape(NDC, 128, FS).transpose(1, 0, 2))
        m["f1b"] = _f32(inp["ff_b1"][li][c * FS:(c + 1) * FS].reshape(NFC, 128).T)
        w2 = inp["ff_W2"][li][c * FS:(c + 1) * FS]       # [FS, D]
        m["f2w"] = _f16(w2.reshape(NFC, 128, D).transpose(1, 0, 2))
        m["f2bT"] = f2bT
        m["lngT"] = lngT
        m["lnbT"] = lnbT
        ow = inp["out_W"][:, c * VSR:(c + 1) * VSR]      # [D, 4000]
        ow = np.concatenate([ow, np.zeros((D, VSP - VSR), np.float32)], axis=1)
        m["outw"] = _f16(ow.reshape(NDC, 128, NVC, 128).transpose(2, 1, 0, 3))
        ob = np.full(VSP, -30.0, np.float32)
        ob[:VSR] = inp["out_b"][c * VSR:(c + 1) * VSR]
        m["outb"] = _f32(ob.reshape(NVC, 128).T)
        in_maps.append(m)
    return in_maps


_NC_CACHE = {}


def kernel(**inputs):
    import os
    inputs = {k: np.asarray(v, dtype=np.float32) for k, v in inputs.items()}
    if "nc" not in _NC_CACHE:
        _NC_CACHE["nc"] = build_bass()
    nc = _NC_CACHE["nc"]
    in_maps = prepare_inputs(inputs)
    trace = bool(int(os.environ.get("KB_TRACE", "0")))
    tmpdir = os.environ.get("KB_TMPDIR") or None
    res = run_bass_kernel_spmd(nc, in_maps, list(range(NCORES)), trace=trace,
                               tmpdir=tmpdir)
    LAST_RESULTS["res"] = res
    shards = [np.asarray(res.results[c]["probsT"][:VSR], dtype=np.float32)
              for c in range(NCORES)]
    return np.ascontiguousarray(np.concatenate(shards, axis=0).T)
